# revision 1
# baseline (speedup 1.0000x reference)
"""MoE layer (top-2 of 8 experts, SwiGLU FFN) on 8 trn2 NeuronCores.

Strategy: expert parallelism. Each core owns one expert. The host computes
only the top-2 *selection* (index lists) and performs the dispatch/combine
data movement (gather tokens per expert / scatter-add partial outputs); all
floating-point math that produces output values — gate logits, top-2
softmax weights, the SwiGLU FFN — runs on device.

Device kernel (identical program on all 8 cores, per-core data):
  inputs   xt    [D, C]  gathered tokens for this expert, transposed
           gw    [D, E]  gate weights, columns rotated so own expert = col 0
           w1,w3 [D, F]  expert FFN in-projections
           w2    [F, D]  expert FFN out-projection
           valid [C]     1.0 for real tokens, 0.0 for padding
  output   yt    [D, C]  weighted expert contribution (transposed)

  per token tile (<=512 tokens):
    logitsT[8, TT] = gw.T @ xT          (PE)
    transpose to [tok, 8], top-2 softmax weight of own expert   (DVE/ACT)
    broadcast weight across partitions via DVE block-transpose + selector
    matmul                                                       (DVE/PE)
    hT[F, TT] = silu(w1.T @ xT) * (w3.T @ xT)                    (PE/ACT/DVE)
    yT[D, TT] = (w2.T)_chunks @ hT, scaled by the gate weight    (PE/DVE)
"""

import numpy as np

T, D, F, E = 8192, 1024, 4096, 8
NCORES = 8
P = 128
TOK_TILE = 512

_nc_cache: dict = {}

# "fp32r": PE multiplies in the hardware's relaxed-fp32 mode (1 cycle/row vs
# 4 for exact fp32), fp32 accumulate in PSUM. "fp32": exact but 4x slower.
MM_MODE = "fp32r"


def _build(C: int, mm_mode: str = MM_MODE):
    """Build + compile the per-core Bass program for capacity C (multiple of 128).

    Token-chunk x F-half blocking: tokens are processed in chunks of up to
    1280 (x and the F-half of hT stay resident in SBUF); for each chunk the
    two F-halves of w1/w3/w2 are streamed exactly once, so total weight
    traffic is one pass per token chunk (~2 passes for C~2304) instead of
    one pass per 512-token tile. The second F-half's output is combined via
    DMA accumulate into the yt DRAM tensor.
    """
    from contextlib import ExitStack

    import concourse.tile as tile
    from concourse import bacc, mybir
    from concourse.bass import ds

    f32 = mybir.dt.float32
    dx = mybir.dt.float32r if mm_mode == "fp32r" else f32
    KD, KF = D // P, F // P
    KH = KF // 2
    X = mybir.AxisListType.X
    Sigmoid = mybir.ActivationFunctionType.Sigmoid
    Exp = mybir.ActivationFunctionType.Exp
    Alu = mybir.AluOpType

    nc = bacc.Bacc(
        "TRN2", target_bir_lowering=False, debug=False, num_devices=NCORES
    )
    xt = nc.dram_tensor("xt", [D, C], dx, kind="ExternalInput")
    gw = nc.dram_tensor("gw", [D, E], dx, kind="ExternalInput")
    w1 = nc.dram_tensor("w1", [D, F], dx, kind="ExternalInput")
    w3 = nc.dram_tensor("w3", [D, F], dx, kind="ExternalInput")
    w2 = nc.dram_tensor("w2", [F, D], dx, kind="ExternalInput")
    vd = nc.dram_tensor("valid", [C], f32, kind="ExternalInput")
    yt = nc.dram_tensor("yt", [D, C], f32, kind="ExternalOutput")

    # chunk plan: token chunks <= 1280, each split into tiles <= 512,
    # sub-512 tile (if any) first within its chunk.
    CHUNK = 1280
    nchunks = -(-C // CHUNK)
    base = (C // nchunks) // P * P
    sizes = [base] * nchunks
    for i in range((C - base * nchunks) // P):
        sizes[i] += P
    chunks = []
    t0 = 0
    for cs in sizes:
        rem = cs % TOK_TILE
        tiles = ([(t0 + cs - rem, rem)] if rem else []) + [
            (t, TOK_TILE) for t in range(t0, t0 + cs - rem, TOK_TILE)
        ]
        chunks.append((t0, cs, tiles))
        t0 += cs

    with ExitStack() as ctx:
        tc = ctx.enter_context(tile.TileContext(nc))
        const = ctx.enter_context(tc.tile_pool(name="const", bufs=1))
        xp = ctx.enter_context(tc.tile_pool(name="xp", bufs=1))
        wp = ctx.enter_context(tc.tile_pool(name="wp", bufs=3))
        hp = ctx.enter_context(tc.tile_pool(name="hp", bufs=1))
        yp = ctx.enter_context(tc.tile_pool(name="yp", bufs=3))
        gp = ctx.enter_context(tc.tile_pool(name="gp", bufs=2))
        psA = ctx.enter_context(tc.tile_pool(name="psA", bufs=2, space="PSUM"))
        psG = ctx.enter_context(tc.tile_pool(name="psG", bufs=1, space="PSUM"))
        psB = ctx.enter_context(tc.tile_pool(name="psB", bufs=3, space="PSUM"))

        # constants
        gw_sb = const.tile([P, KD, E], dx)
        nc.sync.dma_start(gw_sb[:], gw[:, :].rearrange("(ko p) e -> p ko e", p=P))
        valid_sb = const.tile([P, C // P], f32)
        nc.sync.dma_start(valid_sb[:], vd[:].rearrange("(o p) -> p o", p=P))
        # selector row: picks partition 0 of the rhs in the broadcast matmul
        sel_sb = const.tile([32, P], f32)
        nc.vector.memset(sel_sb[:], 0.0)
        nc.vector.memset(sel_sb[0:1, :], 1.0)

        for c0, CS, tiles in chunks:
            x_sb = xp.tile([P, KD, CS], dx, tag="x", name=f"x_{c0}")
            T1 = tiles[0][1]
            r1 = tiles[0][0] - c0
            nc.sync.dma_start(
                x_sb[:, :, ds(r1, T1)],
                xt[:, ds(tiles[0][0], T1)].rearrange("(ko p) t -> p ko t", p=P),
            )
            rest = [(t, TT) for (t, TT) in tiles[1:]]
            for t, TT in rest:
                nc.sync.dma_start(
                    x_sb[:, :, ds(t - c0, TT)],
                    xt[:, ds(t, TT)].rearrange("(ko p) t -> p ko t", p=P),
                )
            wb_all = gp.tile([P, CS], f32, tag="wb_all", name=f"wba_{c0}")

            # ---- gating per tile: top-2 softmax weight of own expert ----
            for t0, TT in tiles:
                S = TT // P
                r0 = t0 - c0
                lt_ps = psG.tile([E, TT], f32, tag="g", name=f"lt_{t0}")
                for kd in range(KD):
                    nc.tensor.matmul(
                        lt_ps[:],
                        gw_sb[:, kd, :],
                        x_sb[:, kd, ds(r0, TT)],
                        start=(kd == 0),
                        stop=(kd == KD - 1),
                    )
                lt32 = gp.tile([32, TT], f32, tag="lt32", name=f"lt32_{t0}")
                nc.vector.memset(lt32[:], 0.0)
                nc.vector.tensor_copy(lt32[0:E, :], lt_ps[:])
                lg = gp.tile([P, S, 32], f32, tag="lg", name=f"lg_{t0}")
                for s in range(S):
                    for j in range(4):
                        nc.vector.transpose(
                            lg[ds(32 * j, 32), s],
                            lt32[:, ds(s * P + 32 * j, 32)],
                        )
                L = lg[:, :, 0:E]
                m1 = gp.tile([P, S, 1], f32, tag="m1", name=f"m1_{t0}")
                nc.vector.reduce_max(m1[:], L, axis=X)
                dd = gp.tile([P, S, E], f32, tag="d", name=f"d_{t0}")
                nc.vector.tensor_tensor(
                    dd[:], L, m1[:].to_broadcast((P, S, E)), Alu.subtract
                )
                msk = gp.tile([P, S, E], f32, tag="msk", name=f"msk_{t0}")
                nc.vector.tensor_scalar(msk[:], dd[:], 0.0, None, Alu.is_ge)
                nc.vector.tensor_scalar(
                    msk[:], msk[:], -100000.0, None, Alu.mult
                )
                nc.vector.tensor_add(msk[:], msk[:], dd[:])
                m2 = gp.tile([P, S, 1], f32, tag="m2", name=f"m2_{t0}")
                nc.vector.reduce_max(m2[:], msk[:], axis=X)
                e2 = gp.tile([P, S, 1], f32, tag="e2", name=f"e2_{t0}")
                nc.scalar.activation(e2[:], m2[:], Exp)
                den = gp.tile([P, S, 1], f32, tag="den", name=f"den_{t0}")
                nc.vector.tensor_scalar(den[:], e2[:], 1.0, None, Alu.add)
                rec = gp.tile([P, S, 1], f32, tag="rec", name=f"rec_{t0}")
                nc.vector.reciprocal(rec[:], den[:])
                e0 = gp.tile([P, S, 1], f32, tag="e0", name=f"e0_{t0}")
                nc.scalar.activation(e0[:], dd[:, :, 0:1], Exp)
                wgt = gp.tile([P, S, 1], f32, tag="wgt", name=f"wgt_{t0}")
                nc.vector.tensor_mul(wgt[:], e0[:], rec[:])

                wb_ps = psG.tile([P, TT], f32, tag="g", name=f"wbps_{t0}")
                for s in range(S):
                    wcol = gp.tile(
                        [P, 32], f32, tag="wcol", name=f"wcol_{t0}_{s}"
                    )
                    nc.vector.memset(wcol[:, 1:32], 0.0)
                    nc.vector.tensor_mul(
                        wcol[:, 0:1],
                        wgt[:, s],
                        valid_sb[:, t0 // P + s, None],
                    )
                    wrt = gp.tile([32, P], f32, tag="wrt", name=f"wrt_{t0}_{s}")
                    for j in range(4):
                        nc.vector.transpose(
                            wrt[:, ds(32 * j, 32)], wcol[ds(32 * j, 32), :]
                        )
                    nc.tensor.matmul(
                        wb_ps[:, ds(s * P, P)],
                        sel_sb[:],
                        wrt[:],
                        start=True,
                        stop=True,
                    )
                nc.vector.tensor_copy(wb_all[:, ds(r0, TT)], wb_ps[:])

            for fh in range(2):
                # ---- phase A: hT(F-half) = silu(w1.T x) * (w3.T x) ----
                h_sb = hp.tile([P, KH, CS], dx, tag="h", name=f"h_{c0}_{fh}")
                for fl in range(KH):
                    f = fh * KH + fl
                    w1_sb = wp.tile(
                        [P, KD, P], dx, tag="w1", name=f"w1_{c0}_{f}"
                    )
                    nc.sync.dma_start(
                        w1_sb[:],
                        w1[:, ds(f * P, P)].rearrange(
                            "(ko p) m -> p ko m", p=P
                        ),
                    )
                    w3_sb = wp.tile(
                        [P, KD, P], dx, tag="w3", name=f"w3_{c0}_{f}"
                    )
                    nc.sync.dma_start(
                        w3_sb[:],
                        w3[:, ds(f * P, P)].rearrange(
                            "(ko p) m -> p ko m", p=P
                        ),
                    )
                    for t0, TT in tiles:
                        r0 = t0 - c0
                        h1 = psA.tile(
                            [P, TT], f32, tag="h1", name=f"ph1_{t0}_{f}"
                        )
                        h3 = psA.tile(
                            [P, TT], f32, tag="h3", name=f"ph3_{t0}_{f}"
                        )
                        for kd in range(KD):
                            nc.tensor.matmul(
                                h1[:],
                                w1_sb[:, kd, :],
                                x_sb[:, kd, ds(r0, TT)],
                                start=(kd == 0),
                                stop=(kd == KD - 1),
                            )
                        for kd in range(KD):
                            nc.tensor.matmul(
                                h3[:],
                                w3_sb[:, kd, :],
                                x_sb[:, kd, ds(r0, TT)],
                                start=(kd == 0),
                                stop=(kd == KD - 1),
                            )
                        sg = gp.tile([P, TT], f32, tag="sg", name=f"sg_{t0}_{f}")
                        nc.scalar.activation(sg[:], h1[:], Sigmoid)
                        s1 = gp.tile([P, TT], f32, tag="s1", name=f"s1_{t0}_{f}")
                        nc.vector.tensor_mul(s1[:], sg[:], h1[:])
                        nc.vector.tensor_mul(
                            h_sb[:, fl, ds(r0, TT)], s1[:], h3[:]
                        )

                # ---- phase B: yT(+=) (w2-half.T @ h) * wb ----
                for dm in range(KD):
                    w2_sb = wp.tile(
                        [P, KH, P], dx, tag="w2", name=f"w2_{c0}_{fh}_{dm}"
                    )
                    nc.sync.dma_start(
                        w2_sb[:],
                        w2[ds(fh * KH * P, KH * P), ds(dm * P, P)].rearrange(
                            "(fo p) m -> p fo m", p=P
                        ),
                    )
                    for t0, TT in tiles:
                        r0 = t0 - c0
                        yps = psB.tile(
                            [P, TT], f32, tag="y", name=f"y_{t0}_{fh}_{dm}"
                        )
                        for fk in range(KH):
                            nc.tensor.matmul(
                                yps[:],
                                w2_sb[:, fk, :],
                                h_sb[:, fk, ds(r0, TT)],
                                start=(fk == 0),
                                stop=(fk == KH - 1),
                            )
                        y_sb = yp.tile(
                            [P, TT], f32, tag="y_sb", name=f"ysb_{t0}_{fh}_{dm}"
                        )
                        nc.vector.tensor_mul(
                            y_sb[:], yps[:], wb_all[:, ds(r0, TT)]
                        )
                        if fh == 0:
                            nc.gpsimd.dma_start(
                                yt[ds(dm * P, P), ds(t0, TT)], y_sb[:]
                            )
                        else:
                            nc.gpsimd.dma_start(
                                yt[ds(dm * P, P), ds(t0, TT)],
                                y_sb[:],
                                accum_op=Alu.add,
                            )

    nc.compile()
    return nc


def _route(x: np.ndarray, gw: np.ndarray):
    """Top-2 expert selection (host; indices only — no output values)."""
    logits = x @ gw
    n = x.shape[0]
    top1 = np.argmax(logits, axis=1)
    l2 = logits.copy()
    l2[np.arange(n), top1] = -np.inf
    top2 = np.argmax(l2, axis=1)
    idx = [
        np.nonzero((top1 == e) | (top2 == e))[0].astype(np.int64)
        for e in range(gw.shape[1])
    ]
    return idx


def kernel(x, gate_w, w1, w2, w3, _trace=False, _trace_cores=None, _result_box=None):
    from concourse.bass_utils import run_bass_kernel_spmd

    x = np.ascontiguousarray(np.asarray(x, dtype=np.float32))
    gw = np.ascontiguousarray(np.asarray(gate_w, dtype=np.float32))
    w1 = np.ascontiguousarray(np.asarray(w1, dtype=np.float32))
    w2 = np.ascontiguousarray(np.asarray(w2, dtype=np.float32))
    w3 = np.ascontiguousarray(np.asarray(w3, dtype=np.float32))
    assert x.shape == (T, D) and gw.shape == (D, E), (x.shape, gw.shape)
    assert w1.shape == (E, D, F) and w3.shape == (E, D, F), (w1.shape,)
    assert w2.shape == (E, F, D), (w2.shape,)

    idx = _route(x, gw)
    maxn = max(len(i) for i in idx)
    C = max(P, -(-maxn // P) * P)

    key = (C, MM_MODE)
    if key not in _nc_cache:
        _nc_cache[key] = _build(C)
    nc = _nc_cache[key]

    rot = np.arange(E)
    in_maps = []
    for e in range(E):
        n = len(idx[e])
        xt = np.zeros((D, C), np.float32)
        xt[:, :n] = x[idx[e]].T
        valid = np.zeros((C,), np.float32)
        valid[:n] = 1.0
        in_maps.append(
            {
                "xt": xt,
                "gw": np.ascontiguousarray(gw[:, (rot + e) % E]),
                "w1": w1[e],
                "w3": w3[e],
                "w2": w2[e],
                "valid": valid,
            }
        )

    res = run_bass_kernel_spmd(
        nc,
        in_maps,
        core_ids=list(range(NCORES)),
        trace=_trace,
        trace_cores=_trace_cores,
    )
    if _result_box is not None:
        _result_box.append(res)

    out = np.zeros((T, D), np.float32)
    for e in range(E):
        n = len(idx[e])
        yt = np.asarray(res.results[e]["yt"])
        out[idx[e]] += yt[:, :n].T
    return out



# revision 2
# speedup vs baseline: 1.0113x; 1.0113x over previous
"""MoE layer (top-2 of 8 experts, SwiGLU FFN) on 8 trn2 NeuronCores.

Strategy: expert parallelism. Each core owns one expert. The host computes
only the top-2 *selection* (index lists) and performs the dispatch/combine
data movement (gather tokens per expert / scatter-add partial outputs); all
floating-point math that produces output values — gate logits, top-2
softmax weights, the SwiGLU FFN — runs on device.

All device math is bf16 (inputs converted host-side, fp32 accumulation in
PSUM): same 1 cycle/row PE speed as fp32r but without fp32r's streaming
penalty, and half the LDWEIGHTS/DMA traffic. Weights are pre-transposed on
the host into the exact SBUF tile layout so every DMA line is >=2KB
contiguous.

Device kernel (identical program on all 8 cores, per-core data):
  inputs   xt    [P, KD, C]       gathered tokens, SBUF layout
           gwt   [P, KD*E]        gate weights, own expert = col 0
           w1t   [KF, P, KD*128]  expert FFN in-proj, tile-f major
           w3t   [KF, P, KD*128]
           w2t   [2, KD, P, KH*128] expert FFN out-proj (per F-half)
           valid [P, C//P]        1.0 for real tokens, 0.0 for padding
  output   yt    [D, C]  weighted expert contribution (transposed)

  gating per token tile (<=512 tokens):
    logitsT[8, TT] = gwt.T @ xT          (PE)
    transpose to [tok, 8], top-2 softmax weight of own expert (DVE/ACT)
    broadcast weight across partitions via DVE block-transpose + selector
    matmul -> wb_all [P, C]              (DVE/PE)
  per F-half:
    hT[KH, C] = silu(w1.T @ xT) * (w3.T @ xT)            (PE/ACT/DVE)
    yT[D, C] (+)= (w2-half.T @ hT) * wb_all              (PE/DVE/DMA)
"""

import numpy as np

T, D, F, E = 8192, 1024, 4096, 8
NCORES = 8
P = 128
TOK_TILE = 512
KD, KF = D // P, F // P
KH = KF // 2

_nc_cache: dict = {}


def _build(C: int):
    """Build + compile the per-core Bass program for capacity C (mult of 128)."""
    from contextlib import ExitStack

    import concourse.tile as tile
    from concourse import bacc, mybir
    from concourse.bass import ds

    f32 = mybir.dt.float32
    bf16 = mybir.dt.bfloat16
    X = mybir.AxisListType.X
    Silu = mybir.ActivationFunctionType.Silu
    Exp = mybir.ActivationFunctionType.Exp
    Alu = mybir.AluOpType

    nc = bacc.Bacc(
        "TRN2", target_bir_lowering=False, debug=False, num_devices=NCORES
    )
    xt = nc.dram_tensor("xt", [P, KD, C], bf16, kind="ExternalInput")
    gw = nc.dram_tensor("gwt", [P, KD * E], bf16, kind="ExternalInput")
    w1 = nc.dram_tensor("w1t", [KF, P, KD * P], bf16, kind="ExternalInput")
    w3 = nc.dram_tensor("w3t", [KF, P, KD * P], bf16, kind="ExternalInput")
    w2 = nc.dram_tensor("w2t", [2, KD, P, KH * P], bf16, kind="ExternalInput")
    vd = nc.dram_tensor("valid", [P, C // P], f32, kind="ExternalInput")
    yt = nc.dram_tensor("yt", [D, C], f32, kind="ExternalOutput")

    # token tiles: sub-512 tile (if any) first
    rem = C % TOK_TILE
    tiles = ([(0, rem)] if rem else []) + [
        (t, TOK_TILE) for t in range(rem, C, TOK_TILE)
    ]

    with ExitStack() as ctx:
        tc = ctx.enter_context(tile.TileContext(nc))
        const = ctx.enter_context(tc.tile_pool(name="const", bufs=1))
        xp = ctx.enter_context(tc.tile_pool(name="xp", bufs=1))
        wp = ctx.enter_context(tc.tile_pool(name="wp", bufs=3))
        hp = ctx.enter_context(tc.tile_pool(name="hp", bufs=1))
        yp = ctx.enter_context(tc.tile_pool(name="yp", bufs=3))
        gp = ctx.enter_context(tc.tile_pool(name="gp", bufs=2))
        psA = ctx.enter_context(tc.tile_pool(name="psA", bufs=2, space="PSUM"))
        psG = ctx.enter_context(tc.tile_pool(name="psG", bufs=1, space="PSUM"))
        psB = ctx.enter_context(tc.tile_pool(name="psB", bufs=3, space="PSUM"))

        # constants
        gw_sb = const.tile([P, KD, E], bf16)
        nc.sync.dma_start(gw_sb[:], gw[:, :].rearrange("p (ko e) -> p ko e", e=E))
        valid_sb = const.tile([P, C // P], f32)
        nc.sync.dma_start(valid_sb[:], vd[:, :])
        # selector row: picks partition 0 of the rhs in the broadcast matmul
        sel_sb = const.tile([32, P], f32)
        nc.vector.memset(sel_sb[:], 0.0)
        nc.vector.memset(sel_sb[0:1, :], 1.0)

        x_sb = xp.tile([P, KD, C], bf16, tag="x", name="x")
        for t0, TT in tiles:
            nc.sync.dma_start(x_sb[:, :, ds(t0, TT)], xt[:, :, ds(t0, TT)])
        wb_all = gp.tile([P, C], f32, tag="wb_all", name="wba")

        # ---- gating per tile: top-2 softmax weight of own expert ----
        for t0, TT in tiles:
            S = TT // P
            lt_ps = psG.tile([E, TT], f32, tag="g", name=f"lt_{t0}")
            for kd in range(KD):
                nc.tensor.matmul(
                    lt_ps[:],
                    gw_sb[:, kd, :],
                    x_sb[:, kd, ds(t0, TT)],
                    start=(kd == 0),
                    stop=(kd == KD - 1),
                )
            lt32 = gp.tile([32, TT], f32, tag="lt32", name=f"lt32_{t0}")
            nc.vector.memset(lt32[:], 0.0)
            nc.vector.tensor_copy(lt32[0:E, :], lt_ps[:])
            lg = gp.tile([P, S, 32], f32, tag="lg", name=f"lg_{t0}")
            for s in range(S):
                for j in range(4):
                    nc.vector.transpose(
                        lg[ds(32 * j, 32), s],
                        lt32[:, ds(s * P + 32 * j, 32)],
                    )
            L = lg[:, :, 0:E]
            m1 = gp.tile([P, S, 1], f32, tag="m1", name=f"m1_{t0}")
            nc.vector.reduce_max(m1[:], L, axis=X)
            dd = gp.tile([P, S, E], f32, tag="d", name=f"d_{t0}")
            nc.vector.tensor_tensor(
                dd[:], L, m1[:].to_broadcast((P, S, E)), Alu.subtract
            )
            msk = gp.tile([P, S, E], f32, tag="msk", name=f"msk_{t0}")
            nc.vector.tensor_scalar(msk[:], dd[:], 0.0, None, Alu.is_ge)
            nc.vector.tensor_scalar(msk[:], msk[:], -100000.0, None, Alu.mult)
            nc.vector.tensor_add(msk[:], msk[:], dd[:])
            m2 = gp.tile([P, S, 1], f32, tag="m2", name=f"m2_{t0}")
            nc.vector.reduce_max(m2[:], msk[:], axis=X)
            e2 = gp.tile([P, S, 1], f32, tag="e2", name=f"e2_{t0}")
            nc.scalar.activation(e2[:], m2[:], Exp)
            den = gp.tile([P, S, 1], f32, tag="den", name=f"den_{t0}")
            nc.vector.tensor_scalar(den[:], e2[:], 1.0, None, Alu.add)
            rec = gp.tile([P, S, 1], f32, tag="rec", name=f"rec_{t0}")
            nc.vector.reciprocal(rec[:], den[:])
            e0 = gp.tile([P, S, 1], f32, tag="e0", name=f"e0_{t0}")
            nc.scalar.activation(e0[:], dd[:, :, 0:1], Exp)
            wgt = gp.tile([P, S, 1], f32, tag="wgt", name=f"wgt_{t0}")
            nc.vector.tensor_mul(wgt[:], e0[:], rec[:])

            wb_ps = psG.tile([P, TT], f32, tag="g", name=f"wbps_{t0}")
            for s in range(S):
                wcol = gp.tile([P, 32], f32, tag="wcol", name=f"wcol_{t0}_{s}")
                nc.vector.memset(wcol[:, 1:32], 0.0)
                nc.vector.tensor_mul(
                    wcol[:, 0:1],
                    wgt[:, s],
                    valid_sb[:, t0 // P + s, None],
                )
                wrt = gp.tile([32, P], f32, tag="wrt", name=f"wrt_{t0}_{s}")
                for j in range(4):
                    nc.vector.transpose(
                        wrt[:, ds(32 * j, 32)], wcol[ds(32 * j, 32), :]
                    )
                nc.tensor.matmul(
                    wb_ps[:, ds(s * P, P)],
                    sel_sb[:],
                    wrt[:],
                    start=True,
                    stop=True,
                )
            nc.vector.tensor_copy(wb_all[:, ds(t0, TT)], wb_ps[:])

        for fh in range(2):
            # ---- phase A: hT(F-half) = silu(w1.T x) * (w3.T x) ----
            h_sb = hp.tile([P, KH, C], bf16, tag="h", name=f"h_{fh}")
            for fl in range(KH):
                f = fh * KH + fl
                w1_sb = wp.tile([P, KD, P], bf16, tag="w1", name=f"w1_{f}")
                nc.sync.dma_start(
                    w1_sb[:], w1[f].rearrange("p (ko m) -> p ko m", m=P)
                )
                w3_sb = wp.tile([P, KD, P], bf16, tag="w3", name=f"w3_{f}")
                nc.sync.dma_start(
                    w3_sb[:], w3[f].rearrange("p (ko m) -> p ko m", m=P)
                )
                for t0, TT in tiles:
                    h1 = psA.tile([P, TT], f32, tag="h1", name=f"ph1_{t0}_{f}")
                    h3 = psA.tile([P, TT], f32, tag="h3", name=f"ph3_{t0}_{f}")
                    for kd in range(KD):
                        nc.tensor.matmul(
                            h1[:],
                            w1_sb[:, kd, :],
                            x_sb[:, kd, ds(t0, TT)],
                            start=(kd == 0),
                            stop=(kd == KD - 1),
                        )
                    for kd in range(KD):
                        nc.tensor.matmul(
                            h3[:],
                            w3_sb[:, kd, :],
                            x_sb[:, kd, ds(t0, TT)],
                            start=(kd == 0),
                            stop=(kd == KD - 1),
                        )
                    s1 = gp.tile([P, TT], f32, tag="s1", name=f"s1_{t0}_{f}")
                    nc.scalar.activation(s1[:], h1[:], Silu)
                    nc.vector.tensor_mul(h_sb[:, fl, ds(t0, TT)], s1[:], h3[:])

            # ---- phase B: yT(+=) (w2-half.T @ h) * wb ----
            for dm in range(KD):
                w2_sb = wp.tile([P, KH, P], bf16, tag="w2", name=f"w2_{fh}_{dm}")
                nc.sync.dma_start(
                    w2_sb[:], w2[fh, dm].rearrange("p (fo m) -> p fo m", m=P)
                )
                for t0, TT in tiles:
                    yps = psB.tile([P, TT], f32, tag="y", name=f"y_{t0}_{fh}_{dm}")
                    for fk in range(KH):
                        nc.tensor.matmul(
                            yps[:],
                            w2_sb[:, fk, :],
                            h_sb[:, fk, ds(t0, TT)],
                            start=(fk == 0),
                            stop=(fk == KH - 1),
                        )
                    y_sb = yp.tile([P, TT], f32, tag="y_sb", name=f"ysb_{t0}_{fh}_{dm}")
                    nc.vector.tensor_mul(y_sb[:], yps[:], wb_all[:, ds(t0, TT)])
                    if fh == 0:
                        nc.gpsimd.dma_start(yt[ds(dm * P, P), ds(t0, TT)], y_sb[:])
                    else:
                        nc.gpsimd.dma_start(
                            yt[ds(dm * P, P), ds(t0, TT)],
                            y_sb[:],
                            accum_op=Alu.add,
                        )

    nc.compile()
    return nc


def _route(x: np.ndarray, gw: np.ndarray):
    """Top-2 expert selection (host; indices only — no output values)."""
    logits = x @ gw
    n = x.shape[0]
    top1 = np.argmax(logits, axis=1)
    l2 = logits.copy()
    l2[np.arange(n), top1] = -np.inf
    top2 = np.argmax(l2, axis=1)
    idx = [
        np.nonzero((top1 == e) | (top2 == e))[0].astype(np.int64)
        for e in range(gw.shape[1])
    ]
    return idx


def kernel(x, gate_w, w1, w2, w3, _trace=False, _trace_cores=None, _result_box=None):
    import ml_dtypes

    from concourse.bass_utils import run_bass_kernel_spmd

    bf16 = ml_dtypes.bfloat16
    x = np.ascontiguousarray(np.asarray(x, dtype=np.float32))
    gw = np.ascontiguousarray(np.asarray(gate_w, dtype=np.float32))
    w1 = np.asarray(w1, dtype=np.float32)
    w2 = np.asarray(w2, dtype=np.float32)
    w3 = np.asarray(w3, dtype=np.float32)
    assert x.shape == (T, D) and gw.shape == (D, E), (x.shape, gw.shape)
    assert w1.shape == (E, D, F) and w3.shape == (E, D, F), (w1.shape,)
    assert w2.shape == (E, F, D), (w2.shape,)

    idx = _route(x, gw)
    maxn = max(len(i) for i in idx)
    C = max(P, -(-maxn // P) * P)

    if C not in _nc_cache:
        _nc_cache[C] = _build(C)
    nc = _nc_cache[C]

    xb = x.astype(bf16)
    rot = np.arange(E)
    in_maps = []
    for e in range(E):
        n = len(idx[e])
        # xt [P, KD, C]: xt[p, kd, t] = x[tok_t, kd*128+p]
        xt = np.zeros((P, KD, C), bf16)
        xt[:, :, :n] = xb[idx[e]].reshape(n, KD, P).transpose(2, 1, 0)
        valid = np.zeros((P, C // P), np.float32)
        vf = valid.reshape(-1, order="F")
        vf[:n] = 1.0
        valid = vf.reshape(P, C // P, order="F")
        # gwt [P, KD*E]: gwt[p, kd*E+e'] = gw[kd*128+p, (e'+e)%E]
        gwr = gw[:, (rot + e) % E].astype(bf16)
        gwt = np.ascontiguousarray(gwr.reshape(KD, P, E).transpose(1, 0, 2)).reshape(
            P, KD * E
        )
        # w1t/w3t [KF, P, KD*P]: w1t[f, p, kd*P+m] = w1[e][kd*128+p, f*128+m]
        w1t = np.ascontiguousarray(
            w1[e].astype(bf16).reshape(KD, P, KF, P).transpose(2, 1, 0, 3)
        ).reshape(KF, P, KD * P)
        w3t = np.ascontiguousarray(
            w3[e].astype(bf16).reshape(KD, P, KF, P).transpose(2, 1, 0, 3)
        ).reshape(KF, P, KD * P)
        # w2t [2, KD, P, KH*P]: w2t[fh, dm, p, fk*P+m] = w2[e][fh*KH*P + fk*P + p, dm*P+m]
        w2t = np.ascontiguousarray(
            w2[e].astype(bf16).reshape(2, KH, P, KD, P).transpose(0, 3, 2, 1, 4)
        ).reshape(2, KD, P, KH * P)
        in_maps.append(
            {
                "xt": xt,
                "gwt": gwt,
                "w1t": w1t,
                "w3t": w3t,
                "w2t": w2t,
                "valid": valid,
            }
        )

    res = run_bass_kernel_spmd(
        nc,
        in_maps,
        core_ids=list(range(NCORES)),
        trace=_trace,
        trace_cores=_trace_cores,
    )
    if _result_box is not None:
        _result_box.append(res)

    out = np.zeros((T, D), np.float32)
    for e in range(E):
        n = len(idx[e])
        yt = np.asarray(res.results[e]["yt"])
        out[idx[e]] += yt[:, :n].T
    return out


# revision 3
# speedup vs baseline: 1.2161x; 1.2026x over previous
"""MoE layer (top-2 of 8 experts, SwiGLU FFN) on 8 trn2 NeuronCores.

Strategy: balanced expert parallelism. Each core has THREE weight segments
(column ranges of fixed compile-time sizes [C-1280, 768, 512]); each segment
is bound to one expert (weights shipped per-core per-segment), so expert
token lists can be split across cores and the per-core column count C drops
from ceil(max_load/128)*128 to 2176 (vs 2048 ideal). The host computes only
the top-2 *selection* and the dispatch/combine data movement; all math that
produces output values (gate logits, top-2 softmax weights, SwiGLU FFN)
runs on device in bf16 (fp32 accumulation in PSUM).

Device kernel (identical program on all 8 cores, per-core data):
  inputs   xt    [P, KD, C]           gathered tokens, SBUF layout
           gwt   [NS, P, KD*E]        gate weights, segment expert = col 0
           w1t   [NS, KF, P, KD*128]  per-segment FFN in-proj
           w3t   [NS, KF, P, KD*128]
           w2t   [NS, 2, KD, P, KH*128] per-segment FFN out-proj
           valid [P, C//P]            1.0 for real tokens
  output   yt    [D, C]  weighted expert contribution (transposed)
"""

import numpy as np

T, D, F, E = 8192, 1024, 4096, 8
NCORES = 8
P = 128
TOK_TILE = 512
KD, KF = D // P, F // P
KH = KF // 2
NS = 3

_nc_cache: dict = {}


def _segs(C):
    return [C - 1280, 768, 512]


def _seg_tiles(C):
    """[(seg, global_t0, TT)] — sub-512 tile first within each segment."""
    out = []
    off = 0
    for s, B in enumerate(_segs(C)):
        rem = B % TOK_TILE
        if rem:
            out.append((s, off, rem))
        for t in range(rem, B, TOK_TILE):
            out.append((s, off + t, TOK_TILE))
        off += B
    return out


def _build(C: int):
    """Build + compile the per-core Bass program (C multiple of 128, >=2176)."""
    from contextlib import ExitStack

    import concourse.tile as tile
    from concourse import bacc, mybir
    from concourse.bass import ds

    f32 = mybir.dt.float32
    bf16 = mybir.dt.bfloat16
    X = mybir.AxisListType.X
    Silu = mybir.ActivationFunctionType.Silu
    Exp = mybir.ActivationFunctionType.Exp
    Alu = mybir.AluOpType

    nc = bacc.Bacc(
        "TRN2", target_bir_lowering=False, debug=False, num_devices=NCORES
    )
    xt = nc.dram_tensor("xt", [P, KD, C], bf16, kind="ExternalInput")
    gw = nc.dram_tensor("gwt", [NS, P, KD * E], bf16, kind="ExternalInput")
    w1 = nc.dram_tensor("w1t", [NS, KF, P, KD * P], bf16, kind="ExternalInput")
    w3 = nc.dram_tensor("w3t", [NS, KF, P, KD * P], bf16, kind="ExternalInput")
    w2 = nc.dram_tensor(
        "w2t", [NS, 2, KD, P, KH * P], bf16, kind="ExternalInput"
    )
    vd = nc.dram_tensor("valid", [P, C // P], f32, kind="ExternalInput")
    yt = nc.dram_tensor("yt", [D, C], f32, kind="ExternalOutput")

    tiles = _seg_tiles(C)

    with ExitStack() as ctx:
        tc = ctx.enter_context(tile.TileContext(nc))
        const = ctx.enter_context(tc.tile_pool(name="const", bufs=1))
        xp = ctx.enter_context(tc.tile_pool(name="xp", bufs=1))
        wp = ctx.enter_context(tc.tile_pool(name="wp", bufs=3))
        hp = ctx.enter_context(tc.tile_pool(name="hp", bufs=1))
        yp = ctx.enter_context(tc.tile_pool(name="yp", bufs=3))
        gp = ctx.enter_context(tc.tile_pool(name="gp", bufs=2))
        psA = ctx.enter_context(tc.tile_pool(name="psA", bufs=2, space="PSUM"))
        psG = ctx.enter_context(tc.tile_pool(name="psG", bufs=1, space="PSUM"))
        psB = ctx.enter_context(tc.tile_pool(name="psB", bufs=3, space="PSUM"))

        # constants
        gw_sb = const.tile([P, NS, KD, E], bf16)
        nc.sync.dma_start(
            gw_sb[:],
            gw[:, :, :].rearrange("ns p (ko e) -> p ns ko e", e=E),
        )
        valid_sb = const.tile([P, C // P], f32)
        nc.sync.dma_start(valid_sb[:], vd[:, :])
        # selector row: picks partition 0 of the rhs in the broadcast matmul
        sel_sb = const.tile([32, P], f32)
        nc.vector.memset(sel_sb[:], 0.0)
        nc.vector.memset(sel_sb[0:1, :], 1.0)

        x_sb = xp.tile([P, KD, C], bf16, tag="x", name="x")
        for _, t0, TT in tiles:
            nc.sync.dma_start(x_sb[:, :, ds(t0, TT)], xt[:, :, ds(t0, TT)])
        wb_all = xp.tile([P, C], f32, tag="wb_all", name="wba")

        # ---- gating per tile: top-2 softmax weight of segment expert ----
        for sg, t0, TT in tiles:
            S = TT // P
            lt_ps = psG.tile([E, TT], f32, tag="g", name=f"lt_{t0}")
            for kd in range(KD):
                nc.tensor.matmul(
                    lt_ps[:],
                    gw_sb[:, sg, kd, :],
                    x_sb[:, kd, ds(t0, TT)],
                    start=(kd == 0),
                    stop=(kd == KD - 1),
                )
            lt32 = gp.tile([32, TT], f32, tag="lt32", name=f"lt32_{t0}")
            nc.vector.memset(lt32[:], 0.0)
            nc.vector.tensor_copy(lt32[0:E, :], lt_ps[:])
            lg = gp.tile([P, S, 32], f32, tag="lg", name=f"lg_{t0}")
            for s in range(S):
                for j in range(4):
                    nc.vector.transpose(
                        lg[ds(32 * j, 32), s],
                        lt32[:, ds(s * P + 32 * j, 32)],
                    )
            L = lg[:, :, 0:E]
            m1 = gp.tile([P, S, 1], f32, tag="m1", name=f"m1_{t0}")
            nc.vector.reduce_max(m1[:], L, axis=X)
            dd = gp.tile([P, S, E], f32, tag="d", name=f"d_{t0}")
            nc.vector.tensor_tensor(
                dd[:], L, m1[:].to_broadcast((P, S, E)), Alu.subtract
            )
            msk = gp.tile([P, S, E], f32, tag="msk", name=f"msk_{t0}")
            nc.vector.tensor_scalar(msk[:], dd[:], 0.0, None, Alu.is_ge)
            nc.vector.tensor_scalar(msk[:], msk[:], -100000.0, None, Alu.mult)
            nc.vector.tensor_add(msk[:], msk[:], dd[:])
            m2 = gp.tile([P, S, 1], f32, tag="m2", name=f"m2_{t0}")
            nc.vector.reduce_max(m2[:], msk[:], axis=X)
            e2 = gp.tile([P, S, 1], f32, tag="e2", name=f"e2_{t0}")
            nc.scalar.activation(e2[:], m2[:], Exp)
            den = gp.tile([P, S, 1], f32, tag="den", name=f"den_{t0}")
            nc.vector.tensor_scalar(den[:], e2[:], 1.0, None, Alu.add)
            rec = gp.tile([P, S, 1], f32, tag="rec", name=f"rec_{t0}")
            nc.vector.reciprocal(rec[:], den[:])
            e0 = gp.tile([P, S, 1], f32, tag="e0", name=f"e0_{t0}")
            nc.scalar.activation(e0[:], dd[:, :, 0:1], Exp)
            wgt = gp.tile([P, S, 1], f32, tag="wgt", name=f"wgt_{t0}")
            nc.vector.tensor_mul(wgt[:], e0[:], rec[:])

            wb_ps = psG.tile([P, TT], f32, tag="g", name=f"wbps_{t0}")
            for s in range(S):
                wcol = gp.tile([P, 32], f32, tag="wcol", name=f"wcol_{t0}_{s}")
                nc.vector.memset(wcol[:, 1:32], 0.0)
                nc.vector.tensor_mul(
                    wcol[:, 0:1],
                    wgt[:, s],
                    valid_sb[:, t0 // P + s, None],
                )
                wrt = gp.tile([32, P], f32, tag="wrt", name=f"wrt_{t0}_{s}")
                for j in range(4):
                    nc.vector.transpose(
                        wrt[:, ds(32 * j, 32)], wcol[ds(32 * j, 32), :]
                    )
                nc.tensor.matmul(
                    wb_ps[:, ds(s * P, P)],
                    sel_sb[:],
                    wrt[:],
                    start=True,
                    stop=True,
                )
            nc.vector.tensor_copy(wb_all[:, ds(t0, TT)], wb_ps[:])

        for fh in range(2):
            # ---- phase A: hT(F-half) = silu(w1.T x) * (w3.T x) ----
            h_sb = hp.tile([P, KH, C], bf16, tag="h", name=f"h_{fh}")
            for sg in range(NS):
                stiles = [t for t in tiles if t[0] == sg]
                for fl in range(KH):
                    f = fh * KH + fl
                    w1_sb = wp.tile(
                        [P, KD, P], bf16, tag="w1", name=f"w1_{sg}_{f}"
                    )
                    nc.sync.dma_start(
                        w1_sb[:], w1[sg, f].rearrange("p (ko m) -> p ko m", m=P)
                    )
                    w3_sb = wp.tile(
                        [P, KD, P], bf16, tag="w3", name=f"w3_{sg}_{f}"
                    )
                    nc.sync.dma_start(
                        w3_sb[:], w3[sg, f].rearrange("p (ko m) -> p ko m", m=P)
                    )
                    for _, t0, TT in stiles:
                        h1 = psA.tile(
                            [P, TT], f32, tag="h1", name=f"ph1_{t0}_{f}"
                        )
                        h3 = psA.tile(
                            [P, TT], f32, tag="h3", name=f"ph3_{t0}_{f}"
                        )
                        for kd in range(KD):
                            nc.tensor.matmul(
                                h1[:],
                                w1_sb[:, kd, :],
                                x_sb[:, kd, ds(t0, TT)],
                                start=(kd == 0),
                                stop=(kd == KD - 1),
                            )
                        for kd in range(KD):
                            nc.tensor.matmul(
                                h3[:],
                                w3_sb[:, kd, :],
                                x_sb[:, kd, ds(t0, TT)],
                                start=(kd == 0),
                                stop=(kd == KD - 1),
                            )
                        s1 = gp.tile([P, TT], f32, tag="s1", name=f"s1_{t0}_{f}")
                        nc.scalar.activation(s1[:], h1[:], Silu)
                        nc.vector.tensor_mul(
                            h_sb[:, fl, ds(t0, TT)], s1[:], h3[:]
                        )

            # ---- phase B: yT(+=) (w2-half.T @ h) * wb ----
            for sg in range(NS):
                stiles = [t for t in tiles if t[0] == sg]
                for dm in range(KD):
                    w2_sb = wp.tile(
                        [P, KH, P], bf16, tag="w2", name=f"w2_{sg}_{fh}_{dm}"
                    )
                    nc.sync.dma_start(
                        w2_sb[:],
                        w2[sg, fh, dm].rearrange("p (fo m) -> p fo m", m=P),
                    )
                    for _, t0, TT in stiles:
                        yps = psB.tile(
                            [P, TT], f32, tag="y", name=f"y_{t0}_{fh}_{dm}"
                        )
                        for fk in range(KH):
                            nc.tensor.matmul(
                                yps[:],
                                w2_sb[:, fk, :],
                                h_sb[:, fk, ds(t0, TT)],
                                start=(fk == 0),
                                stop=(fk == KH - 1),
                            )
                        y_sb = yp.tile(
                            [P, TT], f32, tag="y_sb", name=f"ysb_{t0}_{fh}_{dm}"
                        )
                        nc.vector.tensor_mul(
                            y_sb[:], yps[:], wb_all[:, ds(t0, TT)]
                        )
                        if fh == 0:
                            nc.gpsimd.dma_start(
                                yt[ds(dm * P, P), ds(t0, TT)], y_sb[:]
                            )
                        else:
                            nc.gpsimd.dma_start(
                                yt[ds(dm * P, P), ds(t0, TT)],
                                y_sb[:],
                                accum_op=Alu.add,
                            )

    nc.compile()
    return nc


def _route(x: np.ndarray, gw: np.ndarray):
    """Top-2 expert selection (host; indices only — no output values)."""
    logits = x @ gw
    n = x.shape[0]
    top1 = np.argmax(logits, axis=1)
    l2 = logits.copy()
    l2[np.arange(n), top1] = -np.inf
    top2 = np.argmax(l2, axis=1)
    idx = [
        np.nonzero((top1 == e) | (top2 == e))[0].astype(np.int64)
        for e in range(gw.shape[1])
    ]
    return idx


def _assign(loads):
    """Pack 8 expert loads into 8 cores x 3 segment bins.

    Returns (C, plan) where plan[core][seg] = (expert, tok_lo, tok_hi)
    (token index range within that expert's gathered list; hi>=lo).
    Bin pattern at C=2176 (segs [896, 768, 512]): the biggest expert gets
    two A-bins + a C-bin, the smallest two B-bins + a C-bin, the middle six
    get A+B+C. Falls back to classic one-expert-per-core at larger C.
    """
    order = np.argsort(loads)[::-1]
    C = 2176
    sA, sB, sC = _segs(C)
    big, mids, small = order[0], order[1:7], order[7]
    ok = (
        loads[big] <= 2 * sA + sC
        and all(loads[m] <= sA + sB + sC for m in mids)
        and loads[small] <= 2 * sB + sC
    )
    if not ok:
        # classic: one expert per core, all 3 segments
        C = max(2176, -(-max(loads) // P) * P)
        plan = []
        for e in range(NCORES):
            lo = 0
            segs = []
            for B in _segs(C):
                hi = min(loads[e], lo + B)
                segs.append((e, lo, hi))
                lo = hi
            plan.append(segs)
        return C, plan

    # bins per expert in slot order [A-slots..., B-slots..., C-slot]
    expert_bins = {int(e): [] for e in order}
    # A bins: cores 0,1 -> big; cores 2..7 -> mids
    abin_owner = [big, big] + list(mids)
    bbin_owner = [small, small] + list(mids)
    cbin_owner = [big, small] + list(mids)
    plan = [[None] * NS for _ in range(NCORES)]
    for core in range(NCORES):
        expert_bins[int(abin_owner[core])].append((core, 0, sA))
        expert_bins[int(bbin_owner[core])].append((core, 1, sB))
        expert_bins[int(cbin_owner[core])].append((core, 2, sC))
    for e, bins in expert_bins.items():
        lo = 0
        for core, slot, cap in bins:
            hi = min(int(loads[e]), lo + cap)
            plan[core][slot] = (e, lo, hi)
            lo = hi
        assert lo == loads[e], (e, lo, loads[e])
    return C, plan


def kernel(x, gate_w, w1, w2, w3, _trace=False, _trace_cores=None, _result_box=None):
    import ml_dtypes

    from concourse.bass_utils import run_bass_kernel_spmd

    bf16 = ml_dtypes.bfloat16
    x = np.ascontiguousarray(np.asarray(x, dtype=np.float32))
    gw = np.ascontiguousarray(np.asarray(gate_w, dtype=np.float32))
    w1 = np.asarray(w1, dtype=np.float32)
    w2 = np.asarray(w2, dtype=np.float32)
    w3 = np.asarray(w3, dtype=np.float32)
    assert x.shape == (T, D) and gw.shape == (D, E), (x.shape, gw.shape)
    assert w1.shape == (E, D, F) and w3.shape == (E, D, F), (w1.shape,)
    assert w2.shape == (E, F, D), (w2.shape,)

    idx = _route(x, gw)
    loads = np.array([len(i) for i in idx])
    C, plan = _assign(loads)

    if C not in _nc_cache:
        _nc_cache[C] = _build(C)
    nc = _nc_cache[C]

    xb = x.astype(bf16)
    rot = np.arange(E)
    # per-expert pre-transposed weights (shared across cores/segments)
    w1T = [
        np.ascontiguousarray(
            w1[e].astype(bf16).reshape(KD, P, KF, P).transpose(2, 1, 0, 3)
        ).reshape(KF, P, KD * P)
        for e in range(E)
    ]
    w3T = [
        np.ascontiguousarray(
            w3[e].astype(bf16).reshape(KD, P, KF, P).transpose(2, 1, 0, 3)
        ).reshape(KF, P, KD * P)
        for e in range(E)
    ]
    w2T = [
        np.ascontiguousarray(
            w2[e].astype(bf16).reshape(2, KH, P, KD, P).transpose(0, 3, 2, 1, 4)
        ).reshape(2, KD, P, KH * P)
        for e in range(E)
    ]
    gwT = [
        np.ascontiguousarray(
            gw[:, (rot + e) % E].astype(bf16).reshape(KD, P, E).transpose(1, 0, 2)
        ).reshape(P, KD * E)
        for e in range(E)
    ]

    offs = np.concatenate([[0], np.cumsum(_segs(C))])
    in_maps = []
    for core in range(NCORES):
        xt = np.zeros((P, KD, C), bf16)
        valid_flat = np.zeros(C, np.float32)
        gwt = np.empty((NS, P, KD * E), bf16)
        w1t = np.empty((NS, KF, P, KD * P), bf16)
        w3t = np.empty((NS, KF, P, KD * P), bf16)
        w2t = np.empty((NS, 2, KD, P, KH * P), bf16)
        for s in range(NS):
            e, lo, hi = plan[core][s]
            n = hi - lo
            o = offs[s]
            if n:
                xt[:, :, o : o + n] = (
                    xb[idx[e][lo:hi]].reshape(n, KD, P).transpose(2, 1, 0)
                )
                valid_flat[o : o + n] = 1.0
            gwt[s] = gwT[e]
            w1t[s] = w1T[e]
            w3t[s] = w3T[e]
            w2t[s] = w2T[e]
        valid = np.ascontiguousarray(
            valid_flat.reshape(C // P, P).T
        )  # [P, C//P], token t=(o*P+p) -> [p, o]
        in_maps.append(
            {
                "xt": xt,
                "gwt": gwt,
                "w1t": w1t,
                "w3t": w3t,
                "w2t": w2t,
                "valid": valid,
            }
        )

    res = run_bass_kernel_spmd(
        nc,
        in_maps,
        core_ids=list(range(NCORES)),
        trace=_trace,
        trace_cores=_trace_cores,
    )
    if _result_box is not None:
        _result_box.append(res)

    out = np.zeros((T, D), np.float32)
    for core in range(NCORES):
        yt = np.asarray(res.results[core]["yt"])
        for s in range(NS):
            e, lo, hi = plan[core][s]
            n = hi - lo
            if n:
                o = offs[s]
                out[idx[e][lo:hi]] += yt[:, o : o + n].T
    return out


# revision 6
# speedup vs baseline: 1.2303x; 1.0116x over previous
"""MoE layer (top-2 of 8 experts, SwiGLU FFN) on 8 trn2 NeuronCores.

Strategy: balanced expert parallelism. Each core has THREE weight segments
(column ranges of fixed compile-time sizes [C-1280, 768, 512]); each segment
is bound to one expert (weights shipped per-core per-segment), so expert
token lists can be split across cores and the per-core column count C drops
from ceil(max_load/128)*128 to 2176 (vs 2048 ideal). The host computes only
the top-2 *selection* and the dispatch/combine data movement; all math that
produces output values (gate logits, top-2 softmax weights, SwiGLU FFN)
runs on device in bf16 (fp32 accumulation in PSUM).

Device kernel (identical program on all 8 cores, per-core data):
  inputs   xt    [P, KD, C]           gathered tokens, SBUF layout
           gwt   [NS, P, KD*E]        gate weights, segment expert = col 0
           w1t   [NS, KF, P, KD*128]  per-segment FFN in-proj
           w3t   [NS, KF, P, KD*128]
           w2t   [NS, 2, KD, P, KH*128] per-segment FFN out-proj
           valid [P, C//P]            1.0 for real tokens
  output   yt    [D, C] bf16  weighted expert contribution (transposed)

Pipeline: gate-logit matmuls + DVE softmax chains for all tiles first; the
PE then starts FFN phase A while DVE finishes; the gate-broadcast matmuls
(tiny) are emitted after the first phase-A iteration so they never stall
the PE. The F-half-0 output stays in SBUF (bf16) and is combined with the
F-half-1 output on DVE, so yt is written exactly once (no DMA accumulate).
"""

import numpy as np

T, D, F, E = 8192, 1024, 4096, 8
NCORES = 8
P = 128
TOK_TILE = 512
KD, KF = D // P, F // P
KH = KF // 2
NS = 3

_nc_cache: dict = {}


def _segs(C):
    return [C - 1280, 768, 512]


def _seg_tiles(C):
    """[(seg, global_t0, TT)] — sub-512 tile first within each segment."""
    out = []
    off = 0
    for s, B in enumerate(_segs(C)):
        rem = B % TOK_TILE
        if rem:
            out.append((s, off, rem))
        for t in range(rem, B, TOK_TILE):
            out.append((s, off + t, TOK_TILE))
        off += B
    return out


def _build(C: int):
    """Build + compile the per-core Bass program (C multiple of 128, >=2176)."""
    from contextlib import ExitStack

    import concourse.tile as tile
    from concourse import bacc, mybir
    from concourse.bass import ds

    f32 = mybir.dt.float32
    bf16 = mybir.dt.bfloat16
    X = mybir.AxisListType.X
    Silu = mybir.ActivationFunctionType.Silu
    Exp = mybir.ActivationFunctionType.Exp
    Copy = mybir.ActivationFunctionType.Copy
    Alu = mybir.AluOpType

    nc = bacc.Bacc(
        "TRN2", target_bir_lowering=False, debug=False, num_devices=NCORES
    )
    xt = nc.dram_tensor("xt", [P, KD, C], bf16, kind="ExternalInput")
    gw = nc.dram_tensor("gwt", [NS, P, KD * E], bf16, kind="ExternalInput")
    w1 = nc.dram_tensor("w1t", [NS, KF, P, KD * P], bf16, kind="ExternalInput")
    w3 = nc.dram_tensor("w3t", [NS, KF, P, KD * P], bf16, kind="ExternalInput")
    w2 = nc.dram_tensor(
        "w2t", [NS, 2, KD, P, KH * P], bf16, kind="ExternalInput"
    )
    vd = nc.dram_tensor("valid", [P, C // P], f32, kind="ExternalInput")
    yt = nc.dram_tensor("yt", [D, C], bf16, kind="ExternalOutput")

    tiles = _seg_tiles(C)

    with ExitStack() as ctx:
        tc = ctx.enter_context(tile.TileContext(nc))
        const = ctx.enter_context(tc.tile_pool(name="const", bufs=1))
        xp = ctx.enter_context(tc.tile_pool(name="xp", bufs=1))
        wp = ctx.enter_context(tc.tile_pool(name="wp", bufs=3))
        hp = ctx.enter_context(tc.tile_pool(name="hp", bufs=1))
        yp = ctx.enter_context(tc.tile_pool(name="yp", bufs=2))
        gp = ctx.enter_context(tc.tile_pool(name="gp", bufs=2))
        gq = ctx.enter_context(tc.tile_pool(name="gq", bufs=5))
        psA = ctx.enter_context(tc.tile_pool(name="psA", bufs=2, space="PSUM"))
        psG = ctx.enter_context(tc.tile_pool(name="psG", bufs=1, space="PSUM"))
        psB = ctx.enter_context(tc.tile_pool(name="psB", bufs=2, space="PSUM"))

        # constants
        gw_sb = const.tile([P, NS, KD, E], bf16)
        nc.sync.dma_start(
            gw_sb[:],
            gw[:, :, :].rearrange("ns p (ko e) -> p ns ko e", e=E),
        )
        valid_sb = const.tile([P, C // P], f32)
        nc.sync.dma_start(valid_sb[:], vd[:, :])
        # selector row: picks partition 0 of the rhs in the broadcast matmul
        sel_sb = const.tile([32, P], f32)
        nc.vector.memset(sel_sb[:], 0.0)
        nc.vector.memset(sel_sb[0:1, :], 1.0)

        x_sb = xp.tile([P, KD, C], bf16, tag="x", name="x")
        for _, t0, TT in tiles:
            nc.sync.dma_start(x_sb[:, :, ds(t0, TT)], xt[:, :, ds(t0, TT)])
        wb_all = xp.tile([P, C], f32, tag="wb_all", name="wba")
        y_acc = xp.tile([P, KD, C], bf16, tag="y_acc", name="yacc")

        # ---- gating pass 1: logits + top-2 softmax weight of own expert,
        # ending in the transposed weight row wrt (DVE); no PE stalls ----
        wrt_tiles = {}
        for sg, t0, TT in tiles:
            S = TT // P
            lt_ps = psG.tile([E, TT], f32, tag="lt", name=f"lt_{t0}")
            for kd in range(KD):
                nc.tensor.matmul(
                    lt_ps[:],
                    gw_sb[:, sg, kd, :],
                    x_sb[:, kd, ds(t0, TT)],
                    start=(kd == 0),
                    stop=(kd == KD - 1),
                )
            lt32 = gp.tile([32, TT], f32, tag="lt32", name=f"lt32_{t0}")
            nc.scalar.activation(lt32[0:E, :], lt_ps[:], Copy)
            lg = gp.tile([P, S, 32], f32, tag="lg", name=f"lg_{t0}")
            for s in range(S):
                for j in range(4):
                    nc.vector.transpose(
                        lg[ds(32 * j, 32), s],
                        lt32[:, ds(s * P + 32 * j, 32)],
                    )
            L = lg[:, :, 0:E]
            m1 = gp.tile([P, S, 1], f32, tag="m1", name=f"m1_{t0}")
            nc.vector.reduce_max(m1[:], L, axis=X)
            dd = gp.tile([P, S, E], f32, tag="d", name=f"d_{t0}")
            nc.vector.tensor_tensor(
                dd[:], L, m1[:].to_broadcast((P, S, E)), Alu.subtract
            )
            msk = gp.tile([P, S, E], f32, tag="msk", name=f"msk_{t0}")
            nc.vector.tensor_scalar(msk[:], dd[:], 0.0, None, Alu.is_ge)
            nc.vector.tensor_scalar(msk[:], msk[:], -100000.0, None, Alu.mult)
            nc.vector.tensor_add(msk[:], msk[:], dd[:])
            m2 = gp.tile([P, S, 1], f32, tag="m2", name=f"m2_{t0}")
            nc.vector.reduce_max(m2[:], msk[:], axis=X)
            e2 = gp.tile([P, S, 1], f32, tag="e2", name=f"e2_{t0}")
            nc.scalar.activation(e2[:], m2[:], Exp)
            den = gp.tile([P, S, 1], f32, tag="den", name=f"den_{t0}")
            nc.vector.tensor_scalar(den[:], e2[:], 1.0, None, Alu.add)
            rec = gp.tile([P, S, 1], f32, tag="rec", name=f"rec_{t0}")
            nc.vector.reciprocal(rec[:], den[:])
            e0 = gp.tile([P, S, 1], f32, tag="e0", name=f"e0_{t0}")
            nc.scalar.activation(e0[:], dd[:, :, 0:1], Exp)
            wgt = gq.tile([P, S, 1], f32, tag="wgt", name=f"wgt_{t0}")
            nc.vector.tensor_mul(wgt[:], e0[:], rec[:])
            wrts = []
            for s in range(S):
                wcol = gq.tile([P, 32], f32, tag=f"wcol{s}", name=f"wcol_{t0}_{s}")
                nc.vector.memset(wcol[:, 1:32], 0.0)
                nc.vector.tensor_mul(
                    wcol[:, 0:1],
                    wgt[:, s],
                    valid_sb[:, t0 // P + s, None],
                )
                wrt = gq.tile([32, P], f32, tag=f"wrt{s}", name=f"wrt_{t0}_{s}")
                for j in range(4):
                    nc.vector.transpose(
                        wrt[:, ds(32 * j, 32)], wcol[ds(32 * j, 32), :]
                    )
                wrts.append(wrt)
            wrt_tiles[t0] = wrts

        def emit_gate_broadcast():
            # pass 2: tiny selector matmuls broadcast the per-token weight
            # across partitions into wb_all
            for sg2, t02, TT2 in tiles:
                wb_ps = psG.tile([P, TT2], f32, tag="wb", name=f"wbps_{t02}")
                for s in range(TT2 // P):
                    nc.tensor.matmul(
                        wb_ps[:, ds(s * P, P)],
                        sel_sb[:],
                        wrt_tiles[t02][s][:],
                        start=True,
                        stop=True,
                    )
                nc.vector.tensor_copy(wb_all[:, ds(t02, TT2)], wb_ps[:])

        for fh in range(2):
            # ---- phase A: hT(F-half) = silu(w1.T x) * (w3.T x) ----
            h_sb = hp.tile([P, KH, C], bf16, tag="h", name=f"h_{fh}")
            for sg in range(NS):
                stiles = [t for t in tiles if t[0] == sg]
                for fl in range(KH):
                    f = fh * KH + fl
                    w1_sb = wp.tile(
                        [P, KD, P], bf16, tag="w1", name=f"w1_{sg}_{f}"
                    )
                    nc.sync.dma_start(
                        w1_sb[:], w1[sg, f].rearrange("p (ko m) -> p ko m", m=P)
                    )
                    w3_sb = wp.tile(
                        [P, KD, P], bf16, tag="w3", name=f"w3_{sg}_{f}"
                    )
                    nc.sync.dma_start(
                        w3_sb[:], w3[sg, f].rearrange("p (ko m) -> p ko m", m=P)
                    )
                    for _, t0, TT in stiles:
                        h1 = psA.tile(
                            [P, TT], f32, tag="h1", name=f"ph1_{t0}_{f}"
                        )
                        h3 = psA.tile(
                            [P, TT], f32, tag="h3", name=f"ph3_{t0}_{f}"
                        )
                        for kd in range(KD):
                            nc.tensor.matmul(
                                h1[:],
                                w1_sb[:, kd, :],
                                x_sb[:, kd, ds(t0, TT)],
                                start=(kd == 0),
                                stop=(kd == KD - 1),
                            )
                        for kd in range(KD):
                            nc.tensor.matmul(
                                h3[:],
                                w3_sb[:, kd, :],
                                x_sb[:, kd, ds(t0, TT)],
                                start=(kd == 0),
                                stop=(kd == KD - 1),
                            )
                        s1 = gp.tile([P, TT], f32, tag="s1", name=f"s1_{t0}_{f}")
                        nc.scalar.activation(s1[:], h1[:], Silu)
                        nc.vector.tensor_mul(
                            h_sb[:, fl, ds(t0, TT)], s1[:], h3[:]
                        )
                    if fh == 0 and sg == 0 and fl == 0:
                        emit_gate_broadcast()

            # ---- phase B: yT(+=) (w2-half.T @ h) * wb ----
            for sg in range(NS):
                stiles = [t for t in tiles if t[0] == sg]
                for dm in range(KD):
                    w2_sb = wp.tile(
                        [P, KH, P], bf16, tag="w2", name=f"w2_{sg}_{fh}_{dm}"
                    )
                    nc.sync.dma_start(
                        w2_sb[:],
                        w2[sg, fh, dm].rearrange("p (fo m) -> p fo m", m=P),
                    )
                    for _, t0, TT in stiles:
                        yps = psB.tile(
                            [P, TT], f32, tag="y", name=f"y_{t0}_{fh}_{dm}"
                        )
                        for fk in range(KH):
                            nc.tensor.matmul(
                                yps[:],
                                w2_sb[:, fk, :],
                                h_sb[:, fk, ds(t0, TT)],
                                start=(fk == 0),
                                stop=(fk == KH - 1),
                            )
                        if fh == 0:
                            nc.vector.tensor_mul(
                                y_acc[:, dm, ds(t0, TT)],
                                yps[:],
                                wb_all[:, ds(t0, TT)],
                            )
                        else:
                            tmp = yp.tile(
                                [P, TT], f32, tag="tmp", name=f"yt_{t0}_{dm}"
                            )
                            nc.vector.tensor_mul(
                                tmp[:], yps[:], wb_all[:, ds(t0, TT)]
                            )
                            y_sb = yp.tile(
                                [P, TT], bf16, tag="y_sb", name=f"ysb_{t0}_{dm}"
                            )
                            nc.vector.tensor_add(
                                y_sb[:], tmp[:], y_acc[:, dm, ds(t0, TT)]
                            )
                            eng = (nc.scalar, nc.gpsimd)[dm % 2]
                            eng.dma_start(
                                yt[ds(dm * P, P), ds(t0, TT)], y_sb[:]
                            )

    nc.compile()
    return nc


def _route(x: np.ndarray, gw: np.ndarray):
    """Top-2 expert selection (host; indices only — no output values)."""
    logits = x @ gw
    n = x.shape[0]
    top1 = np.argmax(logits, axis=1)
    l2 = logits.copy()
    l2[np.arange(n), top1] = -np.inf
    top2 = np.argmax(l2, axis=1)
    idx = [
        np.nonzero((top1 == e) | (top2 == e))[0].astype(np.int64)
        for e in range(gw.shape[1])
    ]
    return idx


def _assign(loads):
    """Pack 8 expert loads into 8 cores x 3 segment bins.

    Returns (C, plan) where plan[core][seg] = (expert, tok_lo, tok_hi)
    (token index range within that expert's gathered list; hi>=lo).
    Bin pattern at C=2176 (segs [896, 768, 512]): the biggest expert gets
    two A-bins + a C-bin, the smallest two B-bins + a C-bin, the middle six
    get A+B+C. Falls back to classic one-expert-per-core at larger C.
    """
    order = np.argsort(loads)[::-1]
    C = 2176
    sA, sB, sC = _segs(C)
    big, mids, small = order[0], order[1:7], order[7]
    ok = (
        loads[big] <= 2 * sA + sC
        and all(loads[m] <= sA + sB + sC for m in mids)
        and loads[small] <= 2 * sB + sC
    )
    if not ok:
        # classic: one expert per core, all 3 segments
        C = max(2176, -(-max(loads) // P) * P)
        plan = []
        for e in range(NCORES):
            lo = 0
            segs = []
            for B in _segs(C):
                hi = min(loads[e], lo + B)
                segs.append((e, lo, hi))
                lo = hi
            plan.append(segs)
        return C, plan

    # bins per expert in slot order [A-slots..., B-slots..., C-slot]
    expert_bins = {int(e): [] for e in order}
    abin_owner = [big, big] + list(mids)
    bbin_owner = [small, small] + list(mids)
    cbin_owner = [big, small] + list(mids)
    plan = [[None] * NS for _ in range(NCORES)]
    for core in range(NCORES):
        expert_bins[int(abin_owner[core])].append((core, 0, sA))
        expert_bins[int(bbin_owner[core])].append((core, 1, sB))
        expert_bins[int(cbin_owner[core])].append((core, 2, sC))
    for e, bins in expert_bins.items():
        lo = 0
        for core, slot, cap in bins:
            hi = min(int(loads[e]), lo + cap)
            plan[core][slot] = (e, lo, hi)
            lo = hi
        assert lo == loads[e], (e, lo, loads[e])
    return C, plan


def kernel(x, gate_w, w1, w2, w3, _trace=False, _trace_cores=None, _result_box=None):
    import ml_dtypes

    from concourse.bass_utils import run_bass_kernel_spmd

    bf16 = ml_dtypes.bfloat16
    x = np.ascontiguousarray(np.asarray(x, dtype=np.float32))
    gw = np.ascontiguousarray(np.asarray(gate_w, dtype=np.float32))
    w1 = np.asarray(w1, dtype=np.float32)
    w2 = np.asarray(w2, dtype=np.float32)
    w3 = np.asarray(w3, dtype=np.float32)
    assert x.shape == (T, D) and gw.shape == (D, E), (x.shape, gw.shape)
    assert w1.shape == (E, D, F) and w3.shape == (E, D, F), (w1.shape,)
    assert w2.shape == (E, F, D), (w2.shape,)

    idx = _route(x, gw)
    loads = np.array([len(i) for i in idx])
    C, plan = _assign(loads)

    if C not in _nc_cache:
        _nc_cache[C] = _build(C)
    nc = _nc_cache[C]

    xb = x.astype(bf16)
    rot = np.arange(E)
    # per-expert pre-transposed weights (shared across cores/segments)
    w1T = [
        np.ascontiguousarray(
            w1[e].astype(bf16).reshape(KD, P, KF, P).transpose(2, 1, 0, 3)
        ).reshape(KF, P, KD * P)
        for e in range(E)
    ]
    w3T = [
        np.ascontiguousarray(
            w3[e].astype(bf16).reshape(KD, P, KF, P).transpose(2, 1, 0, 3)
        ).reshape(KF, P, KD * P)
        for e in range(E)
    ]
    w2T = [
        np.ascontiguousarray(
            w2[e].astype(bf16).reshape(2, KH, P, KD, P).transpose(0, 3, 2, 1, 4)
        ).reshape(2, KD, P, KH * P)
        for e in range(E)
    ]
    gwT = [
        np.ascontiguousarray(
            gw[:, (rot + e) % E].astype(bf16).reshape(KD, P, E).transpose(1, 0, 2)
        ).reshape(P, KD * E)
        for e in range(E)
    ]

    offs = np.concatenate([[0], np.cumsum(_segs(C))])
    in_maps = []
    for core in range(NCORES):
        xt = np.zeros((P, KD, C), bf16)
        valid_flat = np.zeros(C, np.float32)
        gwt = np.empty((NS, P, KD * E), bf16)
        w1t = np.empty((NS, KF, P, KD * P), bf16)
        w3t = np.empty((NS, KF, P, KD * P), bf16)
        w2t = np.empty((NS, 2, KD, P, KH * P), bf16)
        for s in range(NS):
            e, lo, hi = plan[core][s]
            n = hi - lo
            o = offs[s]
            if n:
                xt[:, :, o : o + n] = (
                    xb[idx[e][lo:hi]].reshape(n, KD, P).transpose(2, 1, 0)
                )
                valid_flat[o : o + n] = 1.0
            gwt[s] = gwT[e]
            w1t[s] = w1T[e]
            w3t[s] = w3T[e]
            w2t[s] = w2T[e]
        valid = np.ascontiguousarray(
            valid_flat.reshape(C // P, P).T
        )  # [P, C//P], token t=(o*P+p) -> [p, o]
        in_maps.append(
            {
                "xt": xt,
                "gwt": gwt,
                "w1t": w1t,
                "w3t": w3t,
                "w2t": w2t,
                "valid": valid,
            }
        )

    res = run_bass_kernel_spmd(
        nc,
        in_maps,
        core_ids=list(range(NCORES)),
        trace=_trace,
        trace_cores=_trace_cores,
    )
    if _result_box is not None:
        _result_box.append(res)

    out = np.zeros((T, D), np.float32)
    for core in range(NCORES):
        yt = np.asarray(res.results[core]["yt"]).astype(np.float32)
        for s in range(NS):
            e, lo, hi = plan[core][s]
            n = hi - lo
            if n:
                o = offs[s]
                out[idx[e][lo:hi]] += yt[:, o : o + n].T
    return out


# revision 13
# speedup vs baseline: 1.2555x; 1.0205x over previous
"""MoE layer (top-2 of 8 experts, SwiGLU FFN) on 8 trn2 NeuronCores.

Strategy: balanced expert parallelism. Each core has THREE weight segments
(column ranges of fixed compile-time sizes [C-1280, 768, 512]); each segment
is bound to one expert (weights shipped per-core per-segment), so expert
token lists can be split across cores and the per-core column count C drops
from ceil(max_load/128)*128 to 2176 (vs 2048 ideal). The host computes only
the top-2 *selection* and the dispatch/combine data movement; all math that
produces output values (gate logits, top-2 softmax weights, SwiGLU FFN)
runs on device in bf16 (fp32 accumulation in PSUM).

Device kernel (identical program on all 8 cores, per-core data):
  inputs   xt    [P, KD, C]           gathered tokens, SBUF layout
           gwt   [NS, P, KD*E]        gate weights, segment expert = col 0
           w1t   [NS, KF, P, KD*128]  per-segment FFN in-proj
           w3t   [NS, KF, P, KD*128]
           w2t   [NS, 2, KD, P, KH*128] per-segment FFN out-proj
           valid [P, C//P]            1.0 for real tokens
  output   yt    [D, C] bf16  weighted expert contribution (transposed)

Pipeline: gate-logit matmuls + DVE softmax chains for all tiles first; the
PE then starts FFN phase A while DVE finishes; the gate-broadcast matmuls
(tiny) are emitted after the first phase-A iteration so they never stall
the PE. The F-half-0 output stays in SBUF (bf16) and is combined with the
F-half-1 output on DVE, so yt is written exactly once (no DMA accumulate).
"""

import numpy as np

T, D, F, E = 8192, 1024, 4096, 8
NCORES = 8
P = 128
TOK_TILE = 512
KD, KF = D // P, F // P
KH = KF // 2
NS = 3

_nc_cache: dict = {}


def _segs(C):
    return [C - 1280, 768, 512]


def _seg_tiles(C):
    """[(seg, global_t0, TT)] — sub-512 tile first within each segment."""
    out = []
    off = 0
    for s, B in enumerate(_segs(C)):
        rem = B % TOK_TILE
        if rem:
            out.append((s, off, rem))
        for t in range(rem, B, TOK_TILE):
            out.append((s, off + t, TOK_TILE))
        off += B
    return out


def _build(C: int):
    """Build + compile the per-core Bass program (C multiple of 128, >=2176)."""
    from contextlib import ExitStack

    import concourse.tile as tile
    from concourse import bacc, mybir
    from concourse.bass import ds

    f32 = mybir.dt.float32
    bf16 = mybir.dt.bfloat16
    X = mybir.AxisListType.X
    Silu = mybir.ActivationFunctionType.Silu
    Exp = mybir.ActivationFunctionType.Exp
    Copy = mybir.ActivationFunctionType.Copy
    Alu = mybir.AluOpType

    nc = bacc.Bacc(
        "TRN2", target_bir_lowering=False, debug=False, num_devices=NCORES
    )
    xt = nc.dram_tensor("xt", [P, KD, C], bf16, kind="ExternalInput")
    gw = nc.dram_tensor("gwt", [NS, P, KD * E], bf16, kind="ExternalInput")
    w1 = nc.dram_tensor("w1t", [NS, KF, P, KD * P], bf16, kind="ExternalInput")
    w3 = nc.dram_tensor("w3t", [NS, KF, P, KD * P], bf16, kind="ExternalInput")
    w2 = nc.dram_tensor(
        "w2t", [NS, 2, KD, P, KH * P], bf16, kind="ExternalInput"
    )
    vd = nc.dram_tensor("valid", [P, C // P], f32, kind="ExternalInput")
    sc = nc.dram_tensor("selc", [4, 32, P], bf16, kind="ExternalInput")
    yt = nc.dram_tensor("yt", [D, C], bf16, kind="ExternalOutput")

    tiles = _seg_tiles(C)

    with ExitStack() as ctx:
        tc = ctx.enter_context(tile.TileContext(nc))
        const = ctx.enter_context(tc.tile_pool(name="const", bufs=1))
        xp = ctx.enter_context(tc.tile_pool(name="xp", bufs=1))
        wp = ctx.enter_context(tc.tile_pool(name="wp", bufs=3))
        hp = ctx.enter_context(tc.tile_pool(name="hp", bufs=1))
        yp = ctx.enter_context(tc.tile_pool(name="yp", bufs=2))
        gp = ctx.enter_context(tc.tile_pool(name="gp", bufs=2))
        gq = ctx.enter_context(tc.tile_pool(name="gq", bufs=5))
        psA = ctx.enter_context(tc.tile_pool(name="psA", bufs=2, space="PSUM"))
        psG = ctx.enter_context(tc.tile_pool(name="psG", bufs=1, space="PSUM"))
        psB = ctx.enter_context(tc.tile_pool(name="psB", bufs=2, space="PSUM"))

        # constants
        gw_sb = const.tile([P, NS, KD, E], bf16)
        nc.sync.dma_start(
            gw_sb[:],
            gw[:, :, :].rearrange("ns p (ko e) -> p ns ko e", e=E),
        )
        valid_sb = const.tile([P, C // P], f32)
        nc.sync.dma_start(valid_sb[:], vd[:, :])
        # selector tiles: selS[s] picks partition s of the rhs in the
        # broadcast matmul (constant, shipped from host)
        selS = []
        for s in range(4):
            sl = const.tile([32, P], bf16, name=f"sel{s}")
            nc.sync.dma_start(sl[:], sc[s])
            selS.append(sl)

        # smallest tiles first: cheapest x DMA / logit matmul starts the
        # pipeline sooner
        gtiles = sorted(tiles, key=lambda t: t[2])
        x_sb = xp.tile([P, KD, C], bf16, tag="x", name="x")
        for _, t0, TT in gtiles:
            nc.sync.dma_start(x_sb[:, :, ds(t0, TT)], xt[:, :, ds(t0, TT)])
        wb_all = xp.tile([P, C], f32, tag="wb_all", name="wba")
        y_acc = xp.tile([P, KD, C], bf16, tag="y_acc", name="yacc")

        # ---- gating pass 1 (PE): logit matmuls for every tile, copied to
        # SBUF by the Scalar engine; the DVE softmax chains are emitted
        # later, interleaved into phase A, so they never block its h-muls.
        lt_tiles = {}
        for sg, t0, TT in gtiles:
            lt_ps = psG.tile([E, TT], f32, tag="lt", name=f"lt_{t0}")
            for kd in range(KD):
                nc.tensor.matmul(
                    lt_ps[:],
                    gw_sb[:, sg, kd, :],
                    x_sb[:, kd, ds(t0, TT)],
                    start=(kd == 0),
                    stop=(kd == KD - 1),
                )
            lt32 = gq.tile([32, TT], f32, tag="lt32", name=f"lt32_{t0}")
            nc.scalar.activation(lt32[0:E, :], lt_ps[:], Copy)
            lt_tiles[t0] = lt32

        wgtT_tiles = {}

        def emit_gate_chain(t0, TT):
            # DVE/ACT: top-2 softmax weight of segment expert from lt32,
            # ending in wgtT [32, P] (s-index along partitions, token along
            # free) ready for the broadcast matmul.
            S = TT // P
            lt32 = lt_tiles[t0]
            lg = gp.tile([P, S, 32], f32, tag="lg", name=f"lg_{t0}")
            for s in range(S):
                for j in range(4):
                    nc.vector.transpose(
                        lg[ds(32 * j, 32), s],
                        lt32[:, ds(s * P + 32 * j, 32)],
                    )
            L = lg[:, :, 0:E]
            m1 = gp.tile([P, S, 1], f32, tag="m1", name=f"m1_{t0}")
            nc.vector.reduce_max(m1[:], L, axis=X)
            dd = gp.tile([P, S, E], f32, tag="d", name=f"d_{t0}")
            nc.vector.tensor_tensor(
                dd[:], L, m1[:].to_broadcast((P, S, E)), Alu.subtract
            )
            msk = gp.tile([P, S, E], f32, tag="msk", name=f"msk_{t0}")
            nc.vector.tensor_scalar(msk[:], dd[:], 0.0, None, Alu.is_ge)
            nc.vector.tensor_scalar(msk[:], msk[:], -100000.0, None, Alu.mult)
            nc.vector.tensor_add(msk[:], msk[:], dd[:])
            m2 = gp.tile([P, S, 1], f32, tag="m2", name=f"m2_{t0}")
            nc.vector.reduce_max(m2[:], msk[:], axis=X)
            e2 = gp.tile([P, S, 1], f32, tag="e2", name=f"e2_{t0}")
            nc.scalar.activation(e2[:], m2[:], Exp)
            den = gp.tile([P, S, 1], f32, tag="den", name=f"den_{t0}")
            nc.vector.tensor_scalar(den[:], e2[:], 1.0, None, Alu.add)
            rec = gp.tile([P, S, 1], f32, tag="rec", name=f"rec_{t0}")
            nc.vector.reciprocal(rec[:], den[:])
            e0 = gp.tile([P, S, 1], f32, tag="e0", name=f"e0_{t0}")
            nc.scalar.activation(e0[:], dd[:, :, 0:1], Exp)
            wgt = gp.tile([P, S, 1], f32, tag="wgt", name=f"wgt_{t0}")
            nc.vector.tensor_mul(wgt[:], e0[:], rec[:])
            w32 = gq.tile([P, 32], bf16, tag="w32", name=f"w32_{t0}")
            nc.vector.memset(w32[:], 0.0)
            for s in range(S):
                nc.vector.tensor_mul(
                    w32[:, s : s + 1],
                    wgt[:, s],
                    valid_sb[:, t0 // P + s, None],
                )
            wgtT = gq.tile([32, P], bf16, tag="wgtT", name=f"wgtT_{t0}")
            for j in range(4):
                nc.vector.transpose(
                    wgtT[:, ds(32 * j, 32)], w32[ds(32 * j, 32), :]
                )
            wgtT_tiles[t0] = wgtT

        def emit_gate_broadcast():
            # tiny selector matmuls broadcast the per-token weight across
            # partitions into wb_all
            for sg2, t02, TT2 in gtiles:
                wb_ps = psG.tile([P, TT2], f32, tag="wb", name=f"wbps_{t02}")
                for s in range(TT2 // P):
                    nc.tensor.matmul(
                        wb_ps[:, ds(s * P, P)],
                        selS[s][:],
                        wgtT_tiles[t02][:],
                        start=True,
                        stop=True,
                    )
                nc.vector.tensor_copy(wb_all[:, ds(t02, TT2)], wb_ps[:])

        for fh in range(2):
            # ---- phase A: hT(F-half) = silu(w1.T x) * (w3.T x) ----
            h_sb = hp.tile([P, KH, C], bf16, tag="h", name=f"h_{fh}")
            for sg in range(NS):
                stiles = [t for t in tiles if t[0] == sg]
                for fl in range(KH):
                    f = fh * KH + fl
                    w1_sb = wp.tile(
                        [P, KD, P], bf16, tag="w1", name=f"w1_{sg}_{f}"
                    )
                    nc.sync.dma_start(
                        w1_sb[:], w1[sg, f].rearrange("p (ko m) -> p ko m", m=P)
                    )
                    w3_sb = wp.tile(
                        [P, KD, P], bf16, tag="w3", name=f"w3_{sg}_{f}"
                    )
                    nc.sync.dma_start(
                        w3_sb[:], w3[sg, f].rearrange("p (ko m) -> p ko m", m=P)
                    )
                    for _, t0, TT in stiles:
                        h1 = psA.tile(
                            [P, TT], f32, tag="h1", name=f"ph1_{t0}_{f}"
                        )
                        h3 = psA.tile(
                            [P, TT], f32, tag="h3", name=f"ph3_{t0}_{f}"
                        )
                        for kd in range(KD):
                            nc.tensor.matmul(
                                h1[:],
                                w1_sb[:, kd, :],
                                x_sb[:, kd, ds(t0, TT)],
                                start=(kd == 0),
                                stop=(kd == KD - 1),
                            )
                        for kd in range(KD):
                            nc.tensor.matmul(
                                h3[:],
                                w3_sb[:, kd, :],
                                x_sb[:, kd, ds(t0, TT)],
                                start=(kd == 0),
                                stop=(kd == KD - 1),
                            )
                        s1 = gp.tile([P, TT], f32, tag="s1", name=f"s1_{t0}_{f}")
                        nc.scalar.activation(s1[:], h1[:], Silu)
                        nc.vector.tensor_mul(
                            h_sb[:, fl, ds(t0, TT)], s1[:], h3[:]
                        )
                    if fh == 0 and sg == 0:
                        if fl < len(gtiles):
                            emit_gate_chain(gtiles[fl][1], gtiles[fl][2])
                        elif fl == len(gtiles):
                            emit_gate_broadcast()

            # ---- phase B: yT(+=) (w2-half.T @ h) * wb ----
            for sg in range(NS):
                stiles = [t for t in tiles if t[0] == sg]
                for dm in range(KD):
                    w2_sb = wp.tile(
                        [P, KH, P], bf16, tag="w2", name=f"w2_{sg}_{fh}_{dm}"
                    )
                    nc.sync.dma_start(
                        w2_sb[:],
                        w2[sg, fh, dm].rearrange("p (fo m) -> p fo m", m=P),
                    )
                    for _, t0, TT in stiles:
                        yps = psB.tile(
                            [P, TT], f32, tag="y", name=f"y_{t0}_{fh}_{dm}"
                        )
                        for fk in range(KH):
                            nc.tensor.matmul(
                                yps[:],
                                w2_sb[:, fk, :],
                                h_sb[:, fk, ds(t0, TT)],
                                start=(fk == 0),
                                stop=(fk == KH - 1),
                            )
                        if fh == 0:
                            nc.vector.tensor_mul(
                                y_acc[:, dm, ds(t0, TT)],
                                yps[:],
                                wb_all[:, ds(t0, TT)],
                            )
                        else:
                            tmp = yp.tile(
                                [P, TT], f32, tag="tmp", name=f"yt_{t0}_{dm}"
                            )
                            nc.vector.tensor_mul(
                                tmp[:], yps[:], wb_all[:, ds(t0, TT)]
                            )
                            y_sb = yp.tile(
                                [P, TT], bf16, tag="y_sb", name=f"ysb_{t0}_{dm}"
                            )
                            nc.vector.tensor_add(
                                y_sb[:], tmp[:], y_acc[:, dm, ds(t0, TT)]
                            )
                            eng = (nc.scalar, nc.gpsimd)[dm % 2]
                            eng.dma_start(
                                yt[ds(dm * P, P), ds(t0, TT)], y_sb[:]
                            )

    nc.compile()
    return nc


def _route(x: np.ndarray, gw: np.ndarray):
    """Top-2 expert selection (host; indices only — no output values)."""
    logits = x @ gw
    n = x.shape[0]
    top1 = np.argmax(logits, axis=1)
    l2 = logits.copy()
    l2[np.arange(n), top1] = -np.inf
    top2 = np.argmax(l2, axis=1)
    idx = [
        np.nonzero((top1 == e) | (top2 == e))[0].astype(np.int64)
        for e in range(gw.shape[1])
    ]
    return idx


def _assign(loads):
    """Pack 8 expert loads into 8 cores x 3 segment bins.

    Returns (C, plan) where plan[core][seg] = (expert, tok_lo, tok_hi)
    (token index range within that expert's gathered list; hi>=lo).
    Bin pattern at C=2176 (segs [896, 768, 512]): the biggest expert gets
    two A-bins + a C-bin, the smallest two B-bins + a C-bin, the middle six
    get A+B+C. Falls back to classic one-expert-per-core at larger C.
    """
    order = np.argsort(loads)[::-1]
    C = 2176
    sA, sB, sC = _segs(C)
    big, mids, small = order[0], order[1:7], order[7]
    ok = (
        loads[big] <= 2 * sA + sC
        and all(loads[m] <= sA + sB + sC for m in mids)
        and loads[small] <= 2 * sB + sC
    )
    if not ok:
        # classic: one expert per core, all 3 segments
        C = max(2176, -(-max(loads) // P) * P)
        plan = []
        for e in range(NCORES):
            lo = 0
            segs = []
            for B in _segs(C):
                hi = min(loads[e], lo + B)
                segs.append((e, lo, hi))
                lo = hi
            plan.append(segs)
        return C, plan

    # bins per expert in slot order [A-slots..., B-slots..., C-slot]
    expert_bins = {int(e): [] for e in order}
    abin_owner = [big, big] + list(mids)
    bbin_owner = [small, small] + list(mids)
    cbin_owner = [big, small] + list(mids)
    plan = [[None] * NS for _ in range(NCORES)]
    for core in range(NCORES):
        expert_bins[int(abin_owner[core])].append((core, 0, sA))
        expert_bins[int(bbin_owner[core])].append((core, 1, sB))
        expert_bins[int(cbin_owner[core])].append((core, 2, sC))
    for e, bins in expert_bins.items():
        lo = 0
        for core, slot, cap in bins:
            hi = min(int(loads[e]), lo + cap)
            plan[core][slot] = (e, lo, hi)
            lo = hi
        assert lo == loads[e], (e, lo, loads[e])
    return C, plan


def kernel(x, gate_w, w1, w2, w3, _trace=False, _trace_cores=None, _result_box=None):
    import ml_dtypes

    from concourse.bass_utils import run_bass_kernel_spmd

    bf16 = ml_dtypes.bfloat16
    x = np.ascontiguousarray(np.asarray(x, dtype=np.float32))
    gw = np.ascontiguousarray(np.asarray(gate_w, dtype=np.float32))
    w1 = np.asarray(w1, dtype=np.float32)
    w2 = np.asarray(w2, dtype=np.float32)
    w3 = np.asarray(w3, dtype=np.float32)
    assert x.shape == (T, D) and gw.shape == (D, E), (x.shape, gw.shape)
    assert w1.shape == (E, D, F) and w3.shape == (E, D, F), (w1.shape,)
    assert w2.shape == (E, F, D), (w2.shape,)

    idx = _route(x, gw)
    loads = np.array([len(i) for i in idx])
    C, plan = _assign(loads)

    if C not in _nc_cache:
        _nc_cache[C] = _build(C)
    nc = _nc_cache[C]

    xb = x.astype(bf16)
    rot = np.arange(E)
    # per-expert pre-transposed weights (shared across cores/segments)
    w1T = [
        np.ascontiguousarray(
            w1[e].astype(bf16).reshape(KD, P, KF, P).transpose(2, 1, 0, 3)
        ).reshape(KF, P, KD * P)
        for e in range(E)
    ]
    w3T = [
        np.ascontiguousarray(
            w3[e].astype(bf16).reshape(KD, P, KF, P).transpose(2, 1, 0, 3)
        ).reshape(KF, P, KD * P)
        for e in range(E)
    ]
    w2T = [
        np.ascontiguousarray(
            w2[e].astype(bf16).reshape(2, KH, P, KD, P).transpose(0, 3, 2, 1, 4)
        ).reshape(2, KD, P, KH * P)
        for e in range(E)
    ]
    gwT = [
        np.ascontiguousarray(
            gw[:, (rot + e) % E].astype(bf16).reshape(KD, P, E).transpose(1, 0, 2)
        ).reshape(P, KD * E)
        for e in range(E)
    ]

    selc = np.zeros((4, 32, P), bf16)
    for s in range(4):
        selc[s, s, :] = 1.0

    offs = np.concatenate([[0], np.cumsum(_segs(C))])
    in_maps = []
    for core in range(NCORES):
        xt = np.zeros((P, KD, C), bf16)
        valid_flat = np.zeros(C, np.float32)
        gwt = np.empty((NS, P, KD * E), bf16)
        w1t = np.empty((NS, KF, P, KD * P), bf16)
        w3t = np.empty((NS, KF, P, KD * P), bf16)
        w2t = np.empty((NS, 2, KD, P, KH * P), bf16)
        for s in range(NS):
            e, lo, hi = plan[core][s]
            n = hi - lo
            o = offs[s]
            if n:
                xt[:, :, o : o + n] = (
                    xb[idx[e][lo:hi]].reshape(n, KD, P).transpose(2, 1, 0)
                )
                valid_flat[o : o + n] = 1.0
            gwt[s] = gwT[e]
            w1t[s] = w1T[e]
            w3t[s] = w3T[e]
            w2t[s] = w2T[e]
        valid = np.ascontiguousarray(
            valid_flat.reshape(C // P, P).T
        )  # [P, C//P], token t=(o*P+p) -> [p, o]
        in_maps.append(
            {
                "xt": xt,
                "gwt": gwt,
                "w1t": w1t,
                "w3t": w3t,
                "w2t": w2t,
                "valid": valid,
                "selc": selc,
            }
        )

    res = run_bass_kernel_spmd(
        nc,
        in_maps,
        core_ids=list(range(NCORES)),
        trace=_trace,
        trace_cores=_trace_cores,
    )
    if _result_box is not None:
        _result_box.append(res)

    out = np.zeros((T, D), np.float32)
    for core in range(NCORES):
        yt = np.asarray(res.results[core]["yt"]).astype(np.float32)
        for s in range(NS):
            e, lo, hi = plan[core][s]
            n = hi - lo
            if n:
                o = offs[s]
                out[idx[e][lo:hi]] += yt[:, o : o + n].T
    return out


# revision 16
# speedup vs baseline: 1.2558x; 1.0002x over previous
"""MoE layer (top-2 of 8 experts, SwiGLU FFN) on 8 trn2 NeuronCores.

Strategy: balanced expert parallelism. Each core has THREE weight segments
(column ranges of fixed compile-time sizes [C-1280, 768, 512]); each segment
is bound to one expert (weights shipped per-core per-segment), so expert
token lists can be split across cores and the per-core column count C drops
from ceil(max_load/128)*128 to 2176 (vs 2048 ideal). The host computes only
the top-2 *selection* and the dispatch/combine data movement; all math that
produces output values (gate logits, top-2 softmax weights, SwiGLU FFN)
runs on device in bf16 (fp32 accumulation in PSUM).

Device kernel (identical program on all 8 cores, per-core data):
  inputs   xt    [P, KD, C]           gathered tokens, SBUF layout
           gwt   [NS, P, KD*E]        gate weights, segment expert = col 0
           w1t   [NS, KF, P, KD*128]  per-segment FFN in-proj
           w3t   [NS, KF, P, KD*128]
           w2t   [NS, 2, KD, P, KH*128] per-segment FFN out-proj
           valid [P, C//P]            1.0 for real tokens
  output   yt    [D, C] bf16  weighted expert contribution (transposed)

Pipeline: gate-logit matmuls + DVE softmax chains for all tiles first; the
PE then starts FFN phase A while DVE finishes; the gate-broadcast matmuls
(tiny) are emitted after the first phase-A iteration so they never stall
the PE. The F-half-0 output stays in SBUF (bf16) and is combined with the
F-half-1 output on DVE, so yt is written exactly once (no DMA accumulate).
"""

import numpy as np

T, D, F, E = 8192, 1024, 4096, 8
NCORES = 8
P = 128
TOK_TILE = 512
KD, KF = D // P, F // P
KH = KF // 2
NS = 3

_nc_cache: dict = {}


def _segs(C):
    return [C - 1280, 768, 512]


def _seg_tiles(C):
    """[(seg, global_t0, TT)] — sub-512 tile first within each segment."""
    out = []
    off = 0
    for s, B in enumerate(_segs(C)):
        rem = B % TOK_TILE
        if rem:
            out.append((s, off, rem))
        for t in range(rem, B, TOK_TILE):
            out.append((s, off + t, TOK_TILE))
        off += B
    return out


def _build(C: int):
    """Build + compile the per-core Bass program (C multiple of 128, >=2176)."""
    from contextlib import ExitStack

    import concourse.tile as tile
    from concourse import bacc, mybir
    from concourse.bass import ds

    f32 = mybir.dt.float32
    bf16 = mybir.dt.bfloat16
    X = mybir.AxisListType.X
    Silu = mybir.ActivationFunctionType.Silu
    Exp = mybir.ActivationFunctionType.Exp
    Copy = mybir.ActivationFunctionType.Copy
    Alu = mybir.AluOpType

    nc = bacc.Bacc(
        "TRN2", target_bir_lowering=False, debug=False, num_devices=NCORES
    )
    xt = nc.dram_tensor("xt", [P, KD, C], bf16, kind="ExternalInput")
    gw = nc.dram_tensor("gwt", [NS, P, KD * E], bf16, kind="ExternalInput")
    w1 = nc.dram_tensor("w1t", [NS, KF, P, KD * P], bf16, kind="ExternalInput")
    w3 = nc.dram_tensor("w3t", [NS, KF, P, KD * P], bf16, kind="ExternalInput")
    w2 = nc.dram_tensor(
        "w2t", [NS, 2, KD, P, KH * P], bf16, kind="ExternalInput"
    )
    vd = nc.dram_tensor("valid", [P, C // P], f32, kind="ExternalInput")
    sc = nc.dram_tensor("selc", [4, 32, P], bf16, kind="ExternalInput")
    yt = nc.dram_tensor("yt", [D, C], bf16, kind="ExternalOutput")

    tiles = _seg_tiles(C)

    with ExitStack() as ctx:
        tc = ctx.enter_context(tile.TileContext(nc))
        const = ctx.enter_context(tc.tile_pool(name="const", bufs=1))
        xp = ctx.enter_context(tc.tile_pool(name="xp", bufs=1))
        wp = ctx.enter_context(tc.tile_pool(name="wp", bufs=3))
        hp = ctx.enter_context(tc.tile_pool(name="hp", bufs=1))
        yp = ctx.enter_context(tc.tile_pool(name="yp", bufs=2))
        gp = ctx.enter_context(tc.tile_pool(name="gp", bufs=2))
        gq = ctx.enter_context(tc.tile_pool(name="gq", bufs=5))
        psA = ctx.enter_context(tc.tile_pool(name="psA", bufs=2, space="PSUM"))
        psG = ctx.enter_context(tc.tile_pool(name="psG", bufs=1, space="PSUM"))
        psB = ctx.enter_context(tc.tile_pool(name="psB", bufs=2, space="PSUM"))

        # constants
        gw_sb = const.tile([P, NS, KD, E], bf16)
        nc.sync.dma_start(
            gw_sb[:],
            gw[:, :, :].rearrange("ns p (ko e) -> p ns ko e", e=E),
        )
        valid_sb = const.tile([P, C // P], f32)
        nc.sync.dma_start(valid_sb[:], vd[:, :])
        # selector tiles: selS[s] picks partition s of the rhs in the
        # broadcast matmul (constant, shipped from host)
        selS = []
        for s in range(4):
            sl = const.tile([32, P], bf16, name=f"sel{s}")
            nc.sync.dma_start(sl[:], sc[s])
            selS.append(sl)

        # x DMA order: phase A's first tiles (segment 0) lead, so its fl=0
        # iteration is never starved; gating works through tiles in the
        # same order.
        gtiles = [tiles[0], tiles[2], tiles[1]] + tiles[3:]
        x_sb = xp.tile([P, KD, C], bf16, tag="x", name="x")
        for _, t0, TT in gtiles:
            nc.sync.dma_start(x_sb[:, :, ds(t0, TT)], xt[:, :, ds(t0, TT)])
        wb_all = xp.tile([P, C], f32, tag="wb_all", name="wba")
        y_acc = xp.tile([P, KD, C], bf16, tag="y_acc", name="yacc")

        # ---- gating (PE): logit matmuls, copied to SBUF by the Scalar
        # engine; emitted interleaved into phase A (as are the DVE softmax
        # chains) so their PSUM-slot waits hide behind FFN matmuls.
        lt_tiles = {}

        def emit_gate_logits(sg, t0, TT):
            lt_ps = psG.tile([E, TT], f32, tag="lt", name=f"lt_{t0}")
            for kd in range(KD):
                nc.tensor.matmul(
                    lt_ps[:],
                    gw_sb[:, sg, kd, :],
                    x_sb[:, kd, ds(t0, TT)],
                    start=(kd == 0),
                    stop=(kd == KD - 1),
                )
            lt32 = gq.tile([32, TT], f32, tag="lt32", name=f"lt32_{t0}")
            nc.scalar.activation(lt32[0:E, :], lt_ps[:], Copy)
            lt_tiles[t0] = lt32

        wgtT_tiles = {}

        def emit_gate_chain(t0, TT):
            # DVE/ACT: top-2 softmax weight of segment expert from lt32,
            # ending in wgtT [32, P] (s-index along partitions, token along
            # free) ready for the broadcast matmul.
            S = TT // P
            lt32 = lt_tiles[t0]
            lg = gp.tile([P, S, 32], f32, tag="lg", name=f"lg_{t0}")
            for s in range(S):
                for j in range(4):
                    nc.vector.transpose(
                        lg[ds(32 * j, 32), s],
                        lt32[:, ds(s * P + 32 * j, 32)],
                    )
            L = lg[:, :, 0:E]
            m1 = gp.tile([P, S, 1], f32, tag="m1", name=f"m1_{t0}")
            nc.vector.reduce_max(m1[:], L, axis=X)
            dd = gp.tile([P, S, E], f32, tag="d", name=f"d_{t0}")
            nc.vector.tensor_tensor(
                dd[:], L, m1[:].to_broadcast((P, S, E)), Alu.subtract
            )
            msk = gp.tile([P, S, E], f32, tag="msk", name=f"msk_{t0}")
            nc.vector.tensor_scalar(msk[:], dd[:], 0.0, None, Alu.is_ge)
            nc.vector.tensor_scalar(msk[:], msk[:], -100000.0, None, Alu.mult)
            nc.vector.tensor_add(msk[:], msk[:], dd[:])
            m2 = gp.tile([P, S, 1], f32, tag="m2", name=f"m2_{t0}")
            nc.vector.reduce_max(m2[:], msk[:], axis=X)
            e2 = gp.tile([P, S, 1], f32, tag="e2", name=f"e2_{t0}")
            nc.scalar.activation(e2[:], m2[:], Exp)
            den = gp.tile([P, S, 1], f32, tag="den", name=f"den_{t0}")
            nc.vector.tensor_scalar(den[:], e2[:], 1.0, None, Alu.add)
            rec = gp.tile([P, S, 1], f32, tag="rec", name=f"rec_{t0}")
            nc.vector.reciprocal(rec[:], den[:])
            e0 = gp.tile([P, S, 1], f32, tag="e0", name=f"e0_{t0}")
            nc.scalar.activation(e0[:], dd[:, :, 0:1], Exp)
            wgt = gp.tile([P, S, 1], f32, tag="wgt", name=f"wgt_{t0}")
            nc.vector.tensor_mul(wgt[:], e0[:], rec[:])
            w32 = gq.tile([P, 32], bf16, tag="w32", name=f"w32_{t0}")
            nc.vector.memset(w32[:], 0.0)
            for s in range(S):
                nc.vector.tensor_mul(
                    w32[:, s : s + 1],
                    wgt[:, s],
                    valid_sb[:, t0 // P + s, None],
                )
            wgtT = gq.tile([32, P], bf16, tag="wgtT", name=f"wgtT_{t0}")
            for j in range(4):
                nc.vector.transpose(
                    wgtT[:, ds(32 * j, 32)], w32[ds(32 * j, 32), :]
                )
            wgtT_tiles[t0] = wgtT

        def emit_gate_broadcast():
            # tiny selector matmuls broadcast the per-token weight across
            # partitions into wb_all
            for sg2, t02, TT2 in gtiles:
                wb_ps = psG.tile([P, TT2], f32, tag="wb", name=f"wbps_{t02}")
                for s in range(TT2 // P):
                    nc.tensor.matmul(
                        wb_ps[:, ds(s * P, P)],
                        selS[s][:],
                        wgtT_tiles[t02][:],
                        start=True,
                        stop=True,
                    )
                nc.vector.tensor_copy(wb_all[:, ds(t02, TT2)], wb_ps[:])

        # gating emission schedule: two logit matmuls lead (the PE's first
        # work, while x still streams in); the rest slot between phase-A
        # f-iterations of (fh=0, seg=0) so every PSUM/engine wait hides
        # behind FFN matmuls.
        for sg0, t00, TT0 in gtiles[:2]:
            emit_gate_logits(sg0, t00, TT0)
        hooks = {}
        for i, (sg0, t00, TT0) in enumerate(gtiles[2:]):
            hooks.setdefault(i, []).append(("lt", (sg0, t00, TT0)))
        for i, (sg0, t00, TT0) in enumerate(gtiles):
            hooks.setdefault(i + 1, []).append(("chain", (sg0, t00, TT0)))
        hooks.setdefault(10, []).append(("bcast", None))

        for fh in range(2):
            # ---- phase A: hT(F-half) = silu(w1.T x) * (w3.T x) ----
            h_sb = hp.tile([P, KH, C], bf16, tag="h", name=f"h_{fh}")
            for sg in range(NS):
                stiles = [t for t in tiles if t[0] == sg]
                for fl in range(KH):
                    f = fh * KH + fl
                    w1_sb = wp.tile(
                        [P, KD, P], bf16, tag="w1", name=f"w1_{sg}_{f}"
                    )
                    nc.sync.dma_start(
                        w1_sb[:], w1[sg, f].rearrange("p (ko m) -> p ko m", m=P)
                    )
                    w3_sb = wp.tile(
                        [P, KD, P], bf16, tag="w3", name=f"w3_{sg}_{f}"
                    )
                    nc.sync.dma_start(
                        w3_sb[:], w3[sg, f].rearrange("p (ko m) -> p ko m", m=P)
                    )
                    for _, t0, TT in stiles:
                        h1 = psA.tile(
                            [P, TT], f32, tag="h1", name=f"ph1_{t0}_{f}"
                        )
                        h3 = psA.tile(
                            [P, TT], f32, tag="h3", name=f"ph3_{t0}_{f}"
                        )
                        for kd in range(KD):
                            nc.tensor.matmul(
                                h1[:],
                                w1_sb[:, kd, :],
                                x_sb[:, kd, ds(t0, TT)],
                                start=(kd == 0),
                                stop=(kd == KD - 1),
                            )
                        for kd in range(KD):
                            nc.tensor.matmul(
                                h3[:],
                                w3_sb[:, kd, :],
                                x_sb[:, kd, ds(t0, TT)],
                                start=(kd == 0),
                                stop=(kd == KD - 1),
                            )
                        s1 = gp.tile([P, TT], f32, tag="s1", name=f"s1_{t0}_{f}")
                        nc.scalar.activation(s1[:], h1[:], Silu)
                        nc.vector.tensor_mul(
                            h_sb[:, fl, ds(t0, TT)], s1[:], h3[:]
                        )
                    if fh == 0 and sg == 0:
                        for kind, arg in hooks.get(fl, []):
                            if kind == "lt":
                                emit_gate_logits(*arg)
                            elif kind == "chain":
                                emit_gate_chain(arg[1], arg[2])
                            else:
                                emit_gate_broadcast()

            # ---- phase B: yT(+=) (w2-half.T @ h) * wb ----
            for sg in range(NS):
                stiles = [t for t in tiles if t[0] == sg]
                for dm in range(KD):
                    w2_sb = wp.tile(
                        [P, KH, P], bf16, tag="w2", name=f"w2_{sg}_{fh}_{dm}"
                    )
                    nc.sync.dma_start(
                        w2_sb[:],
                        w2[sg, fh, dm].rearrange("p (fo m) -> p fo m", m=P),
                    )
                    for _, t0, TT in stiles:
                        yps = psB.tile(
                            [P, TT], f32, tag="y", name=f"y_{t0}_{fh}_{dm}"
                        )
                        for fk in range(KH):
                            nc.tensor.matmul(
                                yps[:],
                                w2_sb[:, fk, :],
                                h_sb[:, fk, ds(t0, TT)],
                                start=(fk == 0),
                                stop=(fk == KH - 1),
                            )
                        if fh == 0:
                            nc.vector.tensor_mul(
                                y_acc[:, dm, ds(t0, TT)],
                                yps[:],
                                wb_all[:, ds(t0, TT)],
                            )
                        else:
                            tmp = yp.tile(
                                [P, TT], f32, tag="tmp", name=f"yt_{t0}_{dm}"
                            )
                            nc.vector.tensor_mul(
                                tmp[:], yps[:], wb_all[:, ds(t0, TT)]
                            )
                            y_sb = yp.tile(
                                [P, TT], bf16, tag="y_sb", name=f"ysb_{t0}_{dm}"
                            )
                            nc.vector.tensor_add(
                                y_sb[:], tmp[:], y_acc[:, dm, ds(t0, TT)]
                            )
                            eng = (nc.scalar, nc.gpsimd)[dm % 2]
                            eng.dma_start(
                                yt[ds(dm * P, P), ds(t0, TT)], y_sb[:]
                            )

    nc.compile()
    return nc


def _route(x: np.ndarray, gw: np.ndarray):
    """Top-2 expert selection (host; indices only — no output values)."""
    logits = x @ gw
    n = x.shape[0]
    top1 = np.argmax(logits, axis=1)
    l2 = logits.copy()
    l2[np.arange(n), top1] = -np.inf
    top2 = np.argmax(l2, axis=1)
    idx = [
        np.nonzero((top1 == e) | (top2 == e))[0].astype(np.int64)
        for e in range(gw.shape[1])
    ]
    return idx


def _assign(loads):
    """Pack 8 expert loads into 8 cores x 3 segment bins.

    Returns (C, plan) where plan[core][seg] = (expert, tok_lo, tok_hi)
    (token index range within that expert's gathered list; hi>=lo).
    Bin pattern at C=2176 (segs [896, 768, 512]): the biggest expert gets
    two A-bins + a C-bin, the smallest two B-bins + a C-bin, the middle six
    get A+B+C. Falls back to classic one-expert-per-core at larger C.
    """
    order = np.argsort(loads)[::-1]
    C = 2176
    sA, sB, sC = _segs(C)
    big, mids, small = order[0], order[1:7], order[7]
    ok = (
        loads[big] <= 2 * sA + sC
        and all(loads[m] <= sA + sB + sC for m in mids)
        and loads[small] <= 2 * sB + sC
    )
    if not ok:
        # classic: one expert per core, all 3 segments
        C = max(2176, -(-max(loads) // P) * P)
        plan = []
        for e in range(NCORES):
            lo = 0
            segs = []
            for B in _segs(C):
                hi = min(loads[e], lo + B)
                segs.append((e, lo, hi))
                lo = hi
            plan.append(segs)
        return C, plan

    # bins per expert in slot order [A-slots..., B-slots..., C-slot]
    expert_bins = {int(e): [] for e in order}
    abin_owner = [big, big] + list(mids)
    bbin_owner = [small, small] + list(mids)
    cbin_owner = [big, small] + list(mids)
    plan = [[None] * NS for _ in range(NCORES)]
    for core in range(NCORES):
        expert_bins[int(abin_owner[core])].append((core, 0, sA))
        expert_bins[int(bbin_owner[core])].append((core, 1, sB))
        expert_bins[int(cbin_owner[core])].append((core, 2, sC))
    for e, bins in expert_bins.items():
        lo = 0
        for core, slot, cap in bins:
            hi = min(int(loads[e]), lo + cap)
            plan[core][slot] = (e, lo, hi)
            lo = hi
        assert lo == loads[e], (e, lo, loads[e])
    return C, plan


def kernel(x, gate_w, w1, w2, w3, _trace=False, _trace_cores=None, _result_box=None):
    import ml_dtypes

    from concourse.bass_utils import run_bass_kernel_spmd

    bf16 = ml_dtypes.bfloat16
    x = np.ascontiguousarray(np.asarray(x, dtype=np.float32))
    gw = np.ascontiguousarray(np.asarray(gate_w, dtype=np.float32))
    w1 = np.asarray(w1, dtype=np.float32)
    w2 = np.asarray(w2, dtype=np.float32)
    w3 = np.asarray(w3, dtype=np.float32)
    assert x.shape == (T, D) and gw.shape == (D, E), (x.shape, gw.shape)
    assert w1.shape == (E, D, F) and w3.shape == (E, D, F), (w1.shape,)
    assert w2.shape == (E, F, D), (w2.shape,)

    idx = _route(x, gw)
    loads = np.array([len(i) for i in idx])
    C, plan = _assign(loads)

    if C not in _nc_cache:
        _nc_cache[C] = _build(C)
    nc = _nc_cache[C]

    xb = x.astype(bf16)
    rot = np.arange(E)
    # per-expert pre-transposed weights (shared across cores/segments)
    w1T = [
        np.ascontiguousarray(
            w1[e].astype(bf16).reshape(KD, P, KF, P).transpose(2, 1, 0, 3)
        ).reshape(KF, P, KD * P)
        for e in range(E)
    ]
    w3T = [
        np.ascontiguousarray(
            w3[e].astype(bf16).reshape(KD, P, KF, P).transpose(2, 1, 0, 3)
        ).reshape(KF, P, KD * P)
        for e in range(E)
    ]
    w2T = [
        np.ascontiguousarray(
            w2[e].astype(bf16).reshape(2, KH, P, KD, P).transpose(0, 3, 2, 1, 4)
        ).reshape(2, KD, P, KH * P)
        for e in range(E)
    ]
    gwT = [
        np.ascontiguousarray(
            gw[:, (rot + e) % E].astype(bf16).reshape(KD, P, E).transpose(1, 0, 2)
        ).reshape(P, KD * E)
        for e in range(E)
    ]

    selc = np.zeros((4, 32, P), bf16)
    for s in range(4):
        selc[s, s, :] = 1.0

    offs = np.concatenate([[0], np.cumsum(_segs(C))])
    in_maps = []
    for core in range(NCORES):
        xt = np.zeros((P, KD, C), bf16)
        valid_flat = np.zeros(C, np.float32)
        gwt = np.empty((NS, P, KD * E), bf16)
        w1t = np.empty((NS, KF, P, KD * P), bf16)
        w3t = np.empty((NS, KF, P, KD * P), bf16)
        w2t = np.empty((NS, 2, KD, P, KH * P), bf16)
        for s in range(NS):
            e, lo, hi = plan[core][s]
            n = hi - lo
            o = offs[s]
            if n:
                xt[:, :, o : o + n] = (
                    xb[idx[e][lo:hi]].reshape(n, KD, P).transpose(2, 1, 0)
                )
                valid_flat[o : o + n] = 1.0
            gwt[s] = gwT[e]
            w1t[s] = w1T[e]
            w3t[s] = w3T[e]
            w2t[s] = w2T[e]
        valid = np.ascontiguousarray(
            valid_flat.reshape(C // P, P).T
        )  # [P, C//P], token t=(o*P+p) -> [p, o]
        in_maps.append(
            {
                "xt": xt,
                "gwt": gwt,
                "w1t": w1t,
                "w3t": w3t,
                "w2t": w2t,
                "valid": valid,
                "selc": selc,
            }
        )

    res = run_bass_kernel_spmd(
        nc,
        in_maps,
        core_ids=list(range(NCORES)),
        trace=_trace,
        trace_cores=_trace_cores,
    )
    if _result_box is not None:
        _result_box.append(res)

    out = np.zeros((T, D), np.float32)
    for core in range(NCORES):
        yt = np.asarray(res.results[core]["yt"]).astype(np.float32)
        for s in range(NS):
            e, lo, hi = plan[core][s]
            n = hi - lo
            if n:
                o = offs[s]
                out[idx[e][lo:hi]] += yt[:, o : o + n].T
    return out


# revision 23
# speedup vs baseline: 1.2965x; 1.0324x over previous
"""MoE layer (top-2 of 8 experts, SwiGLU FFN) on 8 trn2 NeuronCores.

Strategy: balanced expert parallelism. Each core has THREE weight segments
(column ranges of fixed compile-time sizes [C-1280, 768, 512]); each segment
is bound to one expert (weights shipped per-core per-segment), so expert
token lists can be split across cores and the per-core column count C drops
from ceil(max_load/128)*128 to 2176 (vs 2048 ideal). The host computes only
the top-2 *selection* and the dispatch/combine data movement; all math that
produces output values (gate logits, top-2 softmax weights, SwiGLU FFN)
runs on device in bf16 (fp32 accumulation in PSUM).

Device kernel (identical program on all 8 cores, per-core data):
  inputs   xt    [P, KD, C]           gathered tokens, SBUF layout
           gwt   [NS, P, KD*E]        gate weights, segment expert = col 0
           w1t   [NS, KF, P, KD*128]  per-segment FFN in-proj
           w3t   [NS, KF, P, KD*128]
           w2t   [NS, 2, KD, P, KH*128] per-segment FFN out-proj
           valid [P, C//P]            1.0 for real tokens
  output   yt    [D, C] bf16  weighted expert contribution (transposed)

Pipeline: gate-logit matmuls + DVE softmax chains for all tiles first; the
PE then starts FFN phase A while DVE finishes; the gate-broadcast matmuls
(tiny) are emitted after the first phase-A iteration so they never stall
the PE. The F-half-0 output stays in SBUF (bf16) and is combined with the
F-half-1 output on DVE, so yt is written exactly once (no DMA accumulate).
"""

import numpy as np

T, D, F, E = 8192, 1024, 4096, 8
NCORES = 8
P = 128
TOK_TILE = 512
KD, KF = D // P, F // P
KH = KF // 2
NS = 3

_nc_cache: dict = {}


def _segs(C):
    # C=2112 uses a 448-wide third segment (packs the known loads tighter);
    # other capacities keep 512 so the classic fallback always covers.
    return [896, 768, 448] if C == 2112 else [C - 1280, 768, 512]


def _seg_tiles(C):
    """[(seg, global_t0, TT)] — sub-512 tile first within each segment."""
    out = []
    off = 0
    for s, B in enumerate(_segs(C)):
        rem = B % TOK_TILE
        if rem:
            out.append((s, off, rem))
        for t in range(rem, B, TOK_TILE):
            out.append((s, off + t, TOK_TILE))
        off += B
    return out


def _build(C: int):
    """Build + compile the per-core Bass program (C multiple of 128, >=2176)."""
    from contextlib import ExitStack

    import concourse.tile as tile
    from concourse import bacc, mybir
    from concourse.bass import ds

    f32 = mybir.dt.float32
    bf16 = mybir.dt.bfloat16
    X = mybir.AxisListType.X
    Silu = mybir.ActivationFunctionType.Silu
    Exp = mybir.ActivationFunctionType.Exp
    Copy = mybir.ActivationFunctionType.Copy
    Alu = mybir.AluOpType

    nc = bacc.Bacc(
        "TRN2", target_bir_lowering=False, debug=False, num_devices=NCORES
    )
    xt = nc.dram_tensor("xt", [P, KD, C], bf16, kind="ExternalInput")
    gw = nc.dram_tensor("gwt", [NS, P, KD * E], bf16, kind="ExternalInput")
    w1 = nc.dram_tensor("w1t", [NS, KF, P, KD * P], bf16, kind="ExternalInput")
    w3 = nc.dram_tensor("w3t", [NS, KF, P, KD * P], bf16, kind="ExternalInput")
    w2 = nc.dram_tensor(
        "w2t", [NS, 2, KD, P, KH * P], bf16, kind="ExternalInput"
    )
    NB = -(-C // P)  # valid-mask blocks (C=2112 has a partial 17th block)
    vd = nc.dram_tensor("valid", [P, NB], f32, kind="ExternalInput")
    sc = nc.dram_tensor("selc", [4, 32, P], bf16, kind="ExternalInput")
    yt = nc.dram_tensor("yt", [D, C], bf16, kind="ExternalOutput")

    tiles = _seg_tiles(C)

    with ExitStack() as ctx:
        tc = ctx.enter_context(tile.TileContext(nc))
        const = ctx.enter_context(tc.tile_pool(name="const", bufs=1))
        xp = ctx.enter_context(tc.tile_pool(name="xp", bufs=1))
        wp = ctx.enter_context(tc.tile_pool(name="wp", bufs=3))
        hp = ctx.enter_context(tc.tile_pool(name="hp", bufs=1))
        yp = ctx.enter_context(tc.tile_pool(name="yp", bufs=2))
        gp = ctx.enter_context(tc.tile_pool(name="gp", bufs=2))
        gq = ctx.enter_context(tc.tile_pool(name="gq", bufs=5))
        psA = ctx.enter_context(tc.tile_pool(name="psA", bufs=2, space="PSUM"))
        psG = ctx.enter_context(tc.tile_pool(name="psG", bufs=1, space="PSUM"))
        psB = ctx.enter_context(tc.tile_pool(name="psB", bufs=2, space="PSUM"))

        # constants
        gw_sb = const.tile([P, NS, KD, E], bf16)
        nc.sync.dma_start(
            gw_sb[:],
            gw[:, :, :].rearrange("ns p (ko e) -> p ns ko e", e=E),
        )
        valid_sb = const.tile([P, NB], f32)
        nc.sync.dma_start(valid_sb[:], vd[:, :])
        # selector tiles: selS[s] picks partition s of the rhs in the
        # broadcast matmul (constant, shipped from host)
        selS = []
        for s in range(4):
            sl = const.tile([32, P], bf16, name=f"sel{s}")
            nc.sync.dma_start(sl[:], sc[s])
            selS.append(sl)

        # x DMA order: phase A's first tiles (segment 0) lead, so its fl=0
        # iteration is never starved; gating works through tiles in the
        # same order.
        gtiles = [tiles[0], tiles[2], tiles[1]] + tiles[3:]
        x_sb = xp.tile([P, KD, C], bf16, tag="x", name="x")
        for _, t0, TT in gtiles:
            nc.sync.dma_start(x_sb[:, :, ds(t0, TT)], xt[:, :, ds(t0, TT)])
        wb_all = xp.tile([P, C], f32, tag="wb_all", name="wba")
        y_acc = xp.tile([P, KD, C], bf16, tag="y_acc", name="yacc")

        # ---- gating (PE): logit matmuls, copied to SBUF by the Scalar
        # engine; emitted interleaved into phase A (as are the DVE softmax
        # chains) so their PSUM-slot waits hide behind FFN matmuls.
        lt_tiles = {}

        def emit_gate_logits(sg, t0, TT):
            lt_ps = psG.tile([E, TT], f32, tag="lt", name=f"lt_{t0}")
            for kd in range(KD):
                nc.tensor.matmul(
                    lt_ps[:],
                    gw_sb[:, sg, kd, :],
                    x_sb[:, kd, ds(t0, TT)],
                    start=(kd == 0),
                    stop=(kd == KD - 1),
                )
            lt32 = gq.tile([32, TT], f32, tag="lt32", name=f"lt32_{t0}")
            nc.scalar.activation(lt32[0:E, :], lt_ps[:], Copy)
            lt_tiles[t0] = lt32

        wgtT_tiles = {}

        def _blocks(TT):
            # [(s, block_len)] — last block may be partial (64 for TT=448)
            bl = [(s, P) for s in range(TT // P)]
            if TT % P:
                bl.append((TT // P, TT % P))
            return bl

        def emit_gate_chain(t0, TT):
            # DVE/ACT: top-2 softmax weight of segment expert from lt32,
            # ending in wgtT [32, P] (s-index along partitions, token along
            # free) ready for the broadcast matmul.
            blks = _blocks(TT)
            S = len(blks)
            lt32 = lt_tiles[t0]
            lg = gp.tile([P, S, 32], f32, tag="lg", name=f"lg_{t0}")
            for s, bl in blks:
                for j in range(bl // 32):
                    nc.vector.transpose(
                        lg[ds(32 * j, 32), s],
                        lt32[:, ds(s * P + 32 * j, 32)],
                    )
            L = lg[:, :, 0:E]
            m1 = gp.tile([P, S, 1], f32, tag="m1", name=f"m1_{t0}")
            nc.vector.reduce_max(m1[:], L, axis=X)
            dd = gp.tile([P, S, E], f32, tag="d", name=f"d_{t0}")
            nc.vector.tensor_tensor(
                dd[:], L, m1[:].to_broadcast((P, S, E)), Alu.subtract
            )
            msk = gp.tile([P, S, E], f32, tag="msk", name=f"msk_{t0}")
            nc.vector.tensor_scalar(msk[:], dd[:], 0.0, None, Alu.is_ge)
            nc.vector.tensor_scalar(msk[:], msk[:], -100000.0, None, Alu.mult)
            nc.vector.tensor_add(msk[:], msk[:], dd[:])
            m2 = gp.tile([P, S, 1], f32, tag="m2", name=f"m2_{t0}")
            nc.vector.reduce_max(m2[:], msk[:], axis=X)
            e2 = gp.tile([P, S, 1], f32, tag="e2", name=f"e2_{t0}")
            nc.scalar.activation(e2[:], m2[:], Exp)
            den = gp.tile([P, S, 1], f32, tag="den", name=f"den_{t0}")
            nc.vector.tensor_scalar(den[:], e2[:], 1.0, None, Alu.add)
            rec = gp.tile([P, S, 1], f32, tag="rec", name=f"rec_{t0}")
            nc.vector.reciprocal(rec[:], den[:])
            e0 = gp.tile([P, S, 1], f32, tag="e0", name=f"e0_{t0}")
            nc.scalar.activation(e0[:], dd[:, :, 0:1], Exp)
            wgt = gp.tile([P, S, 1], f32, tag="wgt", name=f"wgt_{t0}")
            nc.vector.tensor_mul(wgt[:], e0[:], rec[:])
            w32 = gq.tile([P, 32], bf16, tag="w32", name=f"w32_{t0}")
            nc.vector.memset(w32[:], 0.0)
            for s, bl in blks:
                # partial block: write only its bl partitions so untouched
                # lanes stay 0 (never NaN) for the transpose below
                nc.vector.tensor_mul(
                    w32[0:bl, s : s + 1],
                    wgt[0:bl, s],
                    valid_sb[0:bl, t0 // P + s, None],
                )
            wgtT = gq.tile([32, P], bf16, tag="wgtT", name=f"wgtT_{t0}")
            for j in range(4):
                nc.vector.transpose(
                    wgtT[:, ds(32 * j, 32)], w32[ds(32 * j, 32), :]
                )
            wgtT_tiles[t0] = wgtT

        def emit_gate_broadcast():
            # tiny selector matmuls broadcast the per-token weight across
            # partitions into wb_all
            for sg2, t02, TT2 in gtiles:
                wb_ps = psG.tile([P, TT2], f32, tag="wb", name=f"wbps_{t02}")
                for s, bl in _blocks(TT2):
                    nc.tensor.matmul(
                        wb_ps[:, ds(s * P, bl)],
                        selS[s][:],
                        wgtT_tiles[t02][:, 0:bl],
                        start=True,
                        stop=True,
                    )
                nc.vector.tensor_copy(wb_all[:, ds(t02, TT2)], wb_ps[:])

        # gating emission schedule: two logit matmuls lead (the PE's first
        # work, while x still streams in); the rest slot between phase-A
        # f-iterations of (fh=0, seg=0) so every PSUM/engine wait hides
        # behind FFN matmuls.
        for sg0, t00, TT0 in gtiles[:2]:
            emit_gate_logits(sg0, t00, TT0)
        hooks = {}
        for i, (sg0, t00, TT0) in enumerate(gtiles[2:]):
            hooks.setdefault(i, []).append(("lt", (sg0, t00, TT0)))
        for i, (sg0, t00, TT0) in enumerate(gtiles):
            hooks.setdefault(i + 1, []).append(("chain", (sg0, t00, TT0)))
        hooks.setdefault(10, []).append(("bcast", None))

        for fh in range(2):
            # ---- phase A: hT(F-half) = silu(w1.T x) * (w3.T x) ----
            h_sb = hp.tile([P, KH, C], bf16, tag="h", name=f"h_{fh}")
            for sg in range(NS):
                stiles = [t for t in tiles if t[0] == sg]
                for fl in range(KH):
                    f = fh * KH + fl
                    w1_sb = wp.tile(
                        [P, KD, P], bf16, tag="w1", name=f"w1_{sg}_{f}"
                    )
                    nc.sync.dma_start(
                        w1_sb[:], w1[sg, f].rearrange("p (ko m) -> p ko m", m=P)
                    )
                    w3_sb = wp.tile(
                        [P, KD, P], bf16, tag="w3", name=f"w3_{sg}_{f}"
                    )
                    nc.sync.dma_start(
                        w3_sb[:], w3[sg, f].rearrange("p (ko m) -> p ko m", m=P)
                    )
                    for _, t0, TT in stiles:
                        h1 = psA.tile(
                            [P, TT], f32, tag="h1", name=f"ph1_{t0}_{f}"
                        )
                        h3 = psA.tile(
                            [P, TT], f32, tag="h3", name=f"ph3_{t0}_{f}"
                        )
                        for kd in range(KD):
                            nc.tensor.matmul(
                                h1[:],
                                w1_sb[:, kd, :],
                                x_sb[:, kd, ds(t0, TT)],
                                start=(kd == 0),
                                stop=(kd == KD - 1),
                            )
                        for kd in range(KD):
                            nc.tensor.matmul(
                                h3[:],
                                w3_sb[:, kd, :],
                                x_sb[:, kd, ds(t0, TT)],
                                start=(kd == 0),
                                stop=(kd == KD - 1),
                            )
                        s1 = gp.tile([P, TT], f32, tag="s1", name=f"s1_{t0}_{f}")
                        nc.scalar.activation(s1[:], h1[:], Silu)
                        nc.vector.tensor_mul(
                            h_sb[:, fl, ds(t0, TT)], s1[:], h3[:]
                        )
                    if fh == 0 and sg == 0:
                        for kind, arg in hooks.get(fl, []):
                            if kind == "lt":
                                emit_gate_logits(*arg)
                            elif kind == "chain":
                                emit_gate_chain(arg[1], arg[2])
                            else:
                                emit_gate_broadcast()

            # ---- phase B: yT(+=) (w2-half.T @ h) * wb ----
            for sg in range(NS):
                stiles = [t for t in tiles if t[0] == sg]
                for dm in range(KD):
                    w2_sb = wp.tile(
                        [P, KH, P], bf16, tag="w2", name=f"w2_{sg}_{fh}_{dm}"
                    )
                    nc.sync.dma_start(
                        w2_sb[:],
                        w2[sg, fh, dm].rearrange("p (fo m) -> p fo m", m=P),
                    )
                    for _, t0, TT in stiles:
                        yps = psB.tile(
                            [P, TT], f32, tag="y", name=f"y_{t0}_{fh}_{dm}"
                        )
                        for fk in range(KH):
                            nc.tensor.matmul(
                                yps[:],
                                w2_sb[:, fk, :],
                                h_sb[:, fk, ds(t0, TT)],
                                start=(fk == 0),
                                stop=(fk == KH - 1),
                            )
                        if fh == 0:
                            nc.vector.tensor_mul(
                                y_acc[:, dm, ds(t0, TT)],
                                yps[:],
                                wb_all[:, ds(t0, TT)],
                            )
                        else:
                            tmp = yp.tile(
                                [P, TT], f32, tag="tmp", name=f"yt_{t0}_{dm}"
                            )
                            nc.vector.tensor_mul(
                                tmp[:], yps[:], wb_all[:, ds(t0, TT)]
                            )
                            y_sb = yp.tile(
                                [P, TT], bf16, tag="y_sb", name=f"ysb_{t0}_{dm}"
                            )
                            nc.vector.tensor_add(
                                y_sb[:], tmp[:], y_acc[:, dm, ds(t0, TT)]
                            )
                            eng = (nc.scalar, nc.gpsimd)[dm % 2]
                            eng.dma_start(
                                yt[ds(dm * P, P), ds(t0, TT)], y_sb[:]
                            )

    nc.compile()
    return nc


def _route(x: np.ndarray, gw: np.ndarray):
    """Top-2 expert selection (host; indices only — no output values)."""
    logits = x @ gw
    n = x.shape[0]
    top1 = np.argmax(logits, axis=1)
    l2 = logits.copy()
    l2[np.arange(n), top1] = -np.inf
    top2 = np.argmax(l2, axis=1)
    idx = [
        np.nonzero((top1 == e) | (top2 == e))[0].astype(np.int64)
        for e in range(gw.shape[1])
    ]
    return idx


def _try_pack(loads, C):
    """Bin pattern at capacity C: the nbig largest experts (load > C) take
    two A-bins + a C-bin each, the nbig smallest take two B-bins + a C-bin,
    the middle 8-2*nbig take A+B+C. Returns plan or None."""
    sA, sB, sC = _segs(C)
    order = [int(e) for e in np.argsort(loads)[::-1]]
    nbig = sum(1 for e in order if loads[e] > C)
    if 2 * nbig > NCORES:
        return None
    bigs = order[:nbig]
    mids = order[nbig : NCORES - nbig]
    smalls = order[NCORES - nbig :]
    if any(loads[e] > 2 * sA + sC for e in bigs):
        return None
    if any(loads[e] > 2 * sB + sC for e in smalls):
        return None
    abin_owner = [e for e in bigs for _ in range(2)] + list(mids)
    bbin_owner = [e for e in smalls for _ in range(2)] + list(mids)
    cbin_owner = bigs + smalls + list(mids)
    expert_bins = {e: [] for e in order}
    for core in range(NCORES):
        expert_bins[abin_owner[core]].append((core, 0, sA))
        expert_bins[bbin_owner[core]].append((core, 1, sB))
        expert_bins[cbin_owner[core]].append((core, 2, sC))
    plan = [[None] * NS for _ in range(NCORES)]
    for e, bins in expert_bins.items():
        lo = 0
        for core, slot, cap in bins:
            hi = min(int(loads[e]), lo + cap)
            plan[core][slot] = (e, lo, hi)
            lo = hi
        if lo != loads[e]:
            return None
    return plan


def _assign(loads):
    """Pack 8 expert loads into 8 cores x 3 segment bins.

    Returns (C, plan) with plan[core][seg] = (expert, tok_lo, tok_hi)
    (token range within that expert's gathered list). Tries the tightest
    capacity first; falls back to classic one-expert-per-core.
    """
    for C in (2112, 2176):
        plan = _try_pack(loads, C)
        if plan is not None:
            return C, plan
    # classic: one expert per core, all 3 segments
    C = max(2176, -(-max(loads) // P) * P)
    plan = []
    for e in range(NCORES):
        lo = 0
        segs = []
        for B in _segs(C):
            hi = min(loads[e], lo + B)
            segs.append((e, lo, hi))
            lo = hi
        plan.append(segs)
    return C, plan


def kernel(x, gate_w, w1, w2, w3, _trace=False, _trace_cores=None, _result_box=None):
    import ml_dtypes

    from concourse.bass_utils import run_bass_kernel_spmd

    bf16 = ml_dtypes.bfloat16
    x = np.ascontiguousarray(np.asarray(x, dtype=np.float32))
    gw = np.ascontiguousarray(np.asarray(gate_w, dtype=np.float32))
    w1 = np.asarray(w1, dtype=np.float32)
    w2 = np.asarray(w2, dtype=np.float32)
    w3 = np.asarray(w3, dtype=np.float32)
    assert x.shape == (T, D) and gw.shape == (D, E), (x.shape, gw.shape)
    assert w1.shape == (E, D, F) and w3.shape == (E, D, F), (w1.shape,)
    assert w2.shape == (E, F, D), (w2.shape,)

    idx = _route(x, gw)
    loads = np.array([len(i) for i in idx])
    C, plan = _assign(loads)

    if C not in _nc_cache:
        _nc_cache[C] = _build(C)
    nc = _nc_cache[C]

    xb = x.astype(bf16)
    rot = np.arange(E)
    # per-expert pre-transposed weights (shared across cores/segments)
    w1T = [
        np.ascontiguousarray(
            w1[e].astype(bf16).reshape(KD, P, KF, P).transpose(2, 1, 0, 3)
        ).reshape(KF, P, KD * P)
        for e in range(E)
    ]
    w3T = [
        np.ascontiguousarray(
            w3[e].astype(bf16).reshape(KD, P, KF, P).transpose(2, 1, 0, 3)
        ).reshape(KF, P, KD * P)
        for e in range(E)
    ]
    w2T = [
        np.ascontiguousarray(
            w2[e].astype(bf16).reshape(2, KH, P, KD, P).transpose(0, 3, 2, 1, 4)
        ).reshape(2, KD, P, KH * P)
        for e in range(E)
    ]
    gwT = [
        np.ascontiguousarray(
            gw[:, (rot + e) % E].astype(bf16).reshape(KD, P, E).transpose(1, 0, 2)
        ).reshape(P, KD * E)
        for e in range(E)
    ]

    selc = np.zeros((4, 32, P), bf16)
    for s in range(4):
        selc[s, s, :] = 1.0

    NB = -(-C // P)
    offs = np.concatenate([[0], np.cumsum(_segs(C))])
    in_maps = []
    for core in range(NCORES):
        xt = np.zeros((P, KD, C), bf16)
        valid_flat = np.zeros(NB * P, np.float32)
        gwt = np.empty((NS, P, KD * E), bf16)
        w1t = np.empty((NS, KF, P, KD * P), bf16)
        w3t = np.empty((NS, KF, P, KD * P), bf16)
        w2t = np.empty((NS, 2, KD, P, KH * P), bf16)
        for s in range(NS):
            e, lo, hi = plan[core][s]
            n = hi - lo
            o = offs[s]
            if n:
                xt[:, :, o : o + n] = (
                    xb[idx[e][lo:hi]].reshape(n, KD, P).transpose(2, 1, 0)
                )
                valid_flat[o : o + n] = 1.0
            gwt[s] = gwT[e]
            w1t[s] = w1T[e]
            w3t[s] = w3T[e]
            w2t[s] = w2T[e]
        valid = np.ascontiguousarray(
            valid_flat.reshape(NB, P).T
        )  # [P, NB], token t=(o*P+p) -> [p, o]
        in_maps.append(
            {
                "xt": xt,
                "gwt": gwt,
                "w1t": w1t,
                "w3t": w3t,
                "w2t": w2t,
                "valid": valid,
                "selc": selc,
            }
        )

    res = run_bass_kernel_spmd(
        nc,
        in_maps,
        core_ids=list(range(NCORES)),
        trace=_trace,
        trace_cores=_trace_cores,
    )
    if _result_box is not None:
        _result_box.append(res)

    out = np.zeros((T, D), np.float32)
    for core in range(NCORES):
        yt = np.asarray(res.results[core]["yt"]).astype(np.float32)
        for s in range(NS):
            e, lo, hi = plan[core][s]
            n = hi - lo
            if n:
                o = offs[s]
                out[idx[e][lo:hi]] += yt[:, o : o + n].T
    return out


# revision 25
# speedup vs baseline: 1.2981x; 1.0012x over previous
"""MoE layer (top-2 of 8 experts, SwiGLU FFN) on 8 trn2 NeuronCores.

Strategy: balanced expert parallelism. Each core has THREE weight segments
(column ranges of fixed compile-time sizes, [896, 768, 448] at C=2112);
each segment is bound to one expert (weights shipped per-core per-segment),
so expert token lists can be split across cores and the per-core column
count C drops from ceil(max_load/128)*128 (~2304) to 2112 (vs 2048 ideal).
The host computes only the top-2 *selection* and the dispatch/combine data
movement; all math that produces output values (gate logits, top-2 softmax
weights, SwiGLU FFN) runs on device in bf16 (fp32 accumulation in PSUM),
which streams the PE at the same 1 row/cycle as fp32r with half the
LDWEIGHTS/DMA traffic and none of fp32r's short-row penalty.

Device kernel (identical program on all 8 cores, per-core data):
  inputs   xt    [P, KD, C]           gathered tokens, SBUF layout
           gwt   [NS, P, KD*E]        gate weights, segment expert = col 0
           w1t   [NS, KF, P, KD*128]  per-segment FFN in-proj
           w3t   [NS, KF, P, KD*128]
           w2t   [NS, 2, KD, P, KH*128] per-segment FFN out-proj
           valid [P, C//P]            1.0 for real tokens
  output   yt    [D, C] bf16  weighted expert contribution (transposed)

Pipeline: gate-logit matmuls + DVE softmax chains for all tiles first; the
PE then starts FFN phase A while DVE finishes; the gate-broadcast matmuls
(tiny) are emitted after the first phase-A iteration so they never stall
the PE. The F-half-0 output stays in SBUF (bf16) and is combined with the
F-half-1 output on DVE, so yt is written exactly once (no DMA accumulate).
"""

import numpy as np

T, D, F, E = 8192, 1024, 4096, 8
NCORES = 8
P = 128
TOK_TILE = 512
KD, KF = D // P, F // P
KH = KF // 2
NS = 3

_nc_cache: dict = {}


def _segs(C):
    # C=2112 uses a 448-wide third segment (packs the known loads tighter);
    # other capacities keep 512 so the classic fallback always covers.
    return [896, 768, 448] if C == 2112 else [C - 1280, 768, 512]


def _seg_tiles(C):
    """[(seg, global_t0, TT)] — sub-512 tile first within each segment."""
    out = []
    off = 0
    for s, B in enumerate(_segs(C)):
        rem = B % TOK_TILE
        if rem:
            out.append((s, off, rem))
        for t in range(rem, B, TOK_TILE):
            out.append((s, off + t, TOK_TILE))
        off += B
    return out


def _build(C: int):
    """Build + compile the per-core Bass program (C multiple of 128, >=2176)."""
    from contextlib import ExitStack

    import concourse.tile as tile
    from concourse import bacc, mybir
    from concourse.bass import ds

    f32 = mybir.dt.float32
    bf16 = mybir.dt.bfloat16
    X = mybir.AxisListType.X
    Silu = mybir.ActivationFunctionType.Silu
    Exp = mybir.ActivationFunctionType.Exp
    Copy = mybir.ActivationFunctionType.Copy
    Alu = mybir.AluOpType

    nc = bacc.Bacc(
        "TRN2", target_bir_lowering=False, debug=False, num_devices=NCORES
    )
    xt = nc.dram_tensor("xt", [P, KD, C], bf16, kind="ExternalInput")
    gw = nc.dram_tensor("gwt", [NS, P, KD * E], bf16, kind="ExternalInput")
    w1 = nc.dram_tensor("w1t", [NS, KF, P, KD * P], bf16, kind="ExternalInput")
    w3 = nc.dram_tensor("w3t", [NS, KF, P, KD * P], bf16, kind="ExternalInput")
    w2 = nc.dram_tensor(
        "w2t", [NS, 2, KD, P, KH * P], bf16, kind="ExternalInput"
    )
    NB = -(-C // P)  # valid-mask blocks (C=2112 has a partial 17th block)
    vd = nc.dram_tensor("valid", [P, NB], f32, kind="ExternalInput")
    sc = nc.dram_tensor("selc", [4, 32, P], bf16, kind="ExternalInput")
    yt = nc.dram_tensor("yt", [D, C], bf16, kind="ExternalOutput")

    tiles = _seg_tiles(C)

    with ExitStack() as ctx:
        tc = ctx.enter_context(tile.TileContext(nc))
        const = ctx.enter_context(tc.tile_pool(name="const", bufs=1))
        xp = ctx.enter_context(tc.tile_pool(name="xp", bufs=1))
        wp = ctx.enter_context(tc.tile_pool(name="wp", bufs=3))
        hp = ctx.enter_context(tc.tile_pool(name="hp", bufs=1))
        yp = ctx.enter_context(tc.tile_pool(name="yp", bufs=2))
        gp = ctx.enter_context(tc.tile_pool(name="gp", bufs=2))
        gq = ctx.enter_context(tc.tile_pool(name="gq", bufs=5))
        psA = ctx.enter_context(tc.tile_pool(name="psA", bufs=2, space="PSUM"))
        psG = ctx.enter_context(tc.tile_pool(name="psG", bufs=1, space="PSUM"))
        psB = ctx.enter_context(tc.tile_pool(name="psB", bufs=2, space="PSUM"))

        # constants
        gw_sb = const.tile([P, NS, KD, E], bf16)
        nc.sync.dma_start(
            gw_sb[:],
            gw[:, :, :].rearrange("ns p (ko e) -> p ns ko e", e=E),
        )
        valid_sb = const.tile([P, NB], f32)
        nc.sync.dma_start(valid_sb[:], vd[:, :])
        # selector tiles: selS[s] picks partition s of the rhs in the
        # broadcast matmul (constant, shipped from host)
        selS = []
        for s in range(4):
            sl = const.tile([32, P], bf16, name=f"sel{s}")
            nc.sync.dma_start(sl[:], sc[s])
            selS.append(sl)

        # x DMA order: phase A's first tiles (segment 0) lead, so its fl=0
        # iteration is never starved; gating works through tiles in the
        # same order.
        gtiles = [tiles[0], tiles[2], tiles[1]] + tiles[3:]
        x_sb = xp.tile([P, KD, C], bf16, tag="x", name="x")
        for _, t0, TT in gtiles:
            nc.sync.dma_start(x_sb[:, :, ds(t0, TT)], xt[:, :, ds(t0, TT)])
        wb_all = xp.tile([P, C], f32, tag="wb_all", name="wba")
        y_acc = xp.tile([P, KD, C], bf16, tag="y_acc", name="yacc")

        # ---- gating (PE): logit matmuls, copied to SBUF by the Scalar
        # engine; emitted interleaved into phase A (as are the DVE softmax
        # chains) so their PSUM-slot waits hide behind FFN matmuls.
        lt_tiles = {}

        def emit_gate_logits(sg, t0, TT):
            lt_ps = psG.tile([E, TT], f32, tag="lt", name=f"lt_{t0}")
            for kd in range(KD):
                nc.tensor.matmul(
                    lt_ps[:],
                    gw_sb[:, sg, kd, :],
                    x_sb[:, kd, ds(t0, TT)],
                    start=(kd == 0),
                    stop=(kd == KD - 1),
                )
            lt32 = gq.tile([32, TT], f32, tag="lt32", name=f"lt32_{t0}")
            nc.scalar.activation(lt32[0:E, :], lt_ps[:], Copy)
            lt_tiles[t0] = lt32

        wgtT_tiles = {}

        def _blocks(TT):
            # [(s, block_len)] — last block may be partial (64 for TT=448)
            bl = [(s, P) for s in range(TT // P)]
            if TT % P:
                bl.append((TT // P, TT % P))
            return bl

        def emit_gate_chain(t0, TT):
            # DVE/ACT: top-2 softmax weight of segment expert from lt32,
            # ending in wgtT [32, P] (s-index along partitions, token along
            # free) ready for the broadcast matmul.
            blks = _blocks(TT)
            S = len(blks)
            lt32 = lt_tiles[t0]
            lg = gp.tile([P, S, 32], f32, tag="lg", name=f"lg_{t0}")
            for s, bl in blks:
                for j in range(bl // 32):
                    nc.vector.transpose(
                        lg[ds(32 * j, 32), s],
                        lt32[:, ds(s * P + 32 * j, 32)],
                    )
            L = lg[:, :, 0:E]
            m1 = gp.tile([P, S, 1], f32, tag="m1", name=f"m1_{t0}")
            nc.vector.reduce_max(m1[:], L, axis=X)
            dd = gp.tile([P, S, E], f32, tag="d", name=f"d_{t0}")
            nc.vector.tensor_tensor(
                dd[:], L, m1[:].to_broadcast((P, S, E)), Alu.subtract
            )
            msk = gp.tile([P, S, E], f32, tag="msk", name=f"msk_{t0}")
            nc.vector.tensor_scalar(msk[:], dd[:], 0.0, None, Alu.is_ge)
            nc.vector.tensor_scalar(msk[:], msk[:], -100000.0, None, Alu.mult)
            nc.vector.tensor_add(msk[:], msk[:], dd[:])
            m2 = gp.tile([P, S, 1], f32, tag="m2", name=f"m2_{t0}")
            nc.vector.reduce_max(m2[:], msk[:], axis=X)
            e2 = gp.tile([P, S, 1], f32, tag="e2", name=f"e2_{t0}")
            nc.scalar.activation(e2[:], m2[:], Exp)
            den = gp.tile([P, S, 1], f32, tag="den", name=f"den_{t0}")
            nc.vector.tensor_scalar(den[:], e2[:], 1.0, None, Alu.add)
            rec = gp.tile([P, S, 1], f32, tag="rec", name=f"rec_{t0}")
            nc.vector.reciprocal(rec[:], den[:])
            e0 = gp.tile([P, S, 1], f32, tag="e0", name=f"e0_{t0}")
            nc.scalar.activation(e0[:], dd[:, :, 0:1], Exp)
            wgt = gp.tile([P, S, 1], f32, tag="wgt", name=f"wgt_{t0}")
            nc.vector.tensor_mul(wgt[:], e0[:], rec[:])
            w32 = gq.tile([P, 32], bf16, tag="w32", name=f"w32_{t0}")
            nc.vector.memset(w32[:], 0.0)
            for s, bl in blks:
                # partial block: write only its bl partitions so untouched
                # lanes stay 0 (never NaN) for the transpose below
                nc.vector.tensor_mul(
                    w32[0:bl, s : s + 1],
                    wgt[0:bl, s],
                    valid_sb[0:bl, t0 // P + s, None],
                )
            wgtT = gq.tile([32, P], bf16, tag="wgtT", name=f"wgtT_{t0}")
            for j in range(4):
                nc.vector.transpose(
                    wgtT[:, ds(32 * j, 32)], w32[ds(32 * j, 32), :]
                )
            wgtT_tiles[t0] = wgtT

        def emit_gate_broadcast():
            # tiny selector matmuls broadcast the per-token weight across
            # partitions into wb_all
            for sg2, t02, TT2 in gtiles:
                wb_ps = psG.tile([P, TT2], f32, tag="wb", name=f"wbps_{t02}")
                for s, bl in _blocks(TT2):
                    nc.tensor.matmul(
                        wb_ps[:, ds(s * P, bl)],
                        selS[s][:],
                        wgtT_tiles[t02][:, 0:bl],
                        start=True,
                        stop=True,
                    )
                nc.vector.tensor_copy(wb_all[:, ds(t02, TT2)], wb_ps[:])

        # gating emission schedule: two logit matmuls lead (the PE's first
        # work, while x still streams in); the rest slot between phase-A
        # f-iterations of (fh=0, seg=0) so every PSUM/engine wait hides
        # behind FFN matmuls.
        for sg0, t00, TT0 in gtiles[:2]:
            emit_gate_logits(sg0, t00, TT0)
        hooks = {}
        for i, (sg0, t00, TT0) in enumerate(gtiles[2:]):
            hooks.setdefault(i, []).append(("lt", (sg0, t00, TT0)))
        for i, (sg0, t00, TT0) in enumerate(gtiles):
            hooks.setdefault(i + 1, []).append(("chain", (sg0, t00, TT0)))
        hooks.setdefault(10, []).append(("bcast", None))

        for fh in range(2):
            # ---- phase A: hT(F-half) = silu(w1.T x) * (w3.T x) ----
            h_sb = hp.tile([P, KH, C], bf16, tag="h", name=f"h_{fh}")
            for sg in range(NS):
                stiles = [t for t in tiles if t[0] == sg]
                for fl in range(KH):
                    f = fh * KH + fl
                    w1_sb = wp.tile(
                        [P, KD, P], bf16, tag="w1", name=f"w1_{sg}_{f}"
                    )
                    nc.sync.dma_start(
                        w1_sb[:], w1[sg, f].rearrange("p (ko m) -> p ko m", m=P)
                    )
                    w3_sb = wp.tile(
                        [P, KD, P], bf16, tag="w3", name=f"w3_{sg}_{f}"
                    )
                    nc.sync.dma_start(
                        w3_sb[:], w3[sg, f].rearrange("p (ko m) -> p ko m", m=P)
                    )
                    for _, t0, TT in stiles:
                        h1 = psA.tile(
                            [P, TT], f32, tag="h1", name=f"ph1_{t0}_{f}"
                        )
                        h3 = psA.tile(
                            [P, TT], f32, tag="h3", name=f"ph3_{t0}_{f}"
                        )
                        for kd in range(KD):
                            nc.tensor.matmul(
                                h1[:],
                                w1_sb[:, kd, :],
                                x_sb[:, kd, ds(t0, TT)],
                                start=(kd == 0),
                                stop=(kd == KD - 1),
                            )
                        for kd in range(KD):
                            nc.tensor.matmul(
                                h3[:],
                                w3_sb[:, kd, :],
                                x_sb[:, kd, ds(t0, TT)],
                                start=(kd == 0),
                                stop=(kd == KD - 1),
                            )
                        s1 = gp.tile([P, TT], f32, tag="s1", name=f"s1_{t0}_{f}")
                        nc.scalar.activation(s1[:], h1[:], Silu)
                        nc.vector.tensor_mul(
                            h_sb[:, fl, ds(t0, TT)], s1[:], h3[:]
                        )
                    if fh == 0 and sg == 0:
                        for kind, arg in hooks.pop(fl, []):
                            if kind == "lt":
                                emit_gate_logits(*arg)
                            elif kind == "chain":
                                emit_gate_chain(arg[1], arg[2])
                            else:
                                emit_gate_broadcast()
            if fh == 0:
                # safety: flush any gating hooks that did not fit into the
                # (fh=0, seg=0) f-iterations (only possible for unusually
                # large tile counts)
                for fl in sorted(hooks):
                    for kind, arg in hooks.pop(fl, []):
                        if kind == "lt":
                            emit_gate_logits(*arg)
                        elif kind == "chain":
                            emit_gate_chain(arg[1], arg[2])
                        else:
                            emit_gate_broadcast()

            # ---- phase B: yT(+=) (w2-half.T @ h) * wb ----
            for sg in range(NS):
                stiles = [t for t in tiles if t[0] == sg]
                for dm in range(KD):
                    w2_sb = wp.tile(
                        [P, KH, P], bf16, tag="w2", name=f"w2_{sg}_{fh}_{dm}"
                    )
                    nc.sync.dma_start(
                        w2_sb[:],
                        w2[sg, fh, dm].rearrange("p (fo m) -> p fo m", m=P),
                    )
                    for _, t0, TT in stiles:
                        yps = psB.tile(
                            [P, TT], f32, tag="y", name=f"y_{t0}_{fh}_{dm}"
                        )
                        for fk in range(KH):
                            nc.tensor.matmul(
                                yps[:],
                                w2_sb[:, fk, :],
                                h_sb[:, fk, ds(t0, TT)],
                                start=(fk == 0),
                                stop=(fk == KH - 1),
                            )
                        if fh == 0:
                            nc.vector.tensor_mul(
                                y_acc[:, dm, ds(t0, TT)],
                                yps[:],
                                wb_all[:, ds(t0, TT)],
                            )
                        else:
                            tmp = yp.tile(
                                [P, TT], f32, tag="tmp", name=f"yt_{t0}_{dm}"
                            )
                            nc.vector.tensor_mul(
                                tmp[:], yps[:], wb_all[:, ds(t0, TT)]
                            )
                            y_sb = yp.tile(
                                [P, TT], bf16, tag="y_sb", name=f"ysb_{t0}_{dm}"
                            )
                            nc.vector.tensor_add(
                                y_sb[:], tmp[:], y_acc[:, dm, ds(t0, TT)]
                            )
                            eng = (nc.scalar, nc.gpsimd)[dm % 2]
                            eng.dma_start(
                                yt[ds(dm * P, P), ds(t0, TT)], y_sb[:]
                            )

    nc.compile()
    return nc


def _route(x: np.ndarray, gw: np.ndarray):
    """Top-2 expert selection (host; indices only — no output values)."""
    logits = x @ gw
    n = x.shape[0]
    top1 = np.argmax(logits, axis=1)
    l2 = logits.copy()
    l2[np.arange(n), top1] = -np.inf
    top2 = np.argmax(l2, axis=1)
    idx = [
        np.nonzero((top1 == e) | (top2 == e))[0].astype(np.int64)
        for e in range(gw.shape[1])
    ]
    return idx


def _try_pack(loads, C):
    """Bin pattern at capacity C: the nbig largest experts (load > C) take
    two A-bins + a C-bin each, the nbig smallest take two B-bins + a C-bin,
    the middle 8-2*nbig take A+B+C. Returns plan or None."""
    sA, sB, sC = _segs(C)
    order = [int(e) for e in np.argsort(loads)[::-1]]
    nbig = sum(1 for e in order if loads[e] > C)
    if 2 * nbig > NCORES:
        return None
    bigs = order[:nbig]
    mids = order[nbig : NCORES - nbig]
    smalls = order[NCORES - nbig :]
    if any(loads[e] > 2 * sA + sC for e in bigs):
        return None
    if any(loads[e] > 2 * sB + sC for e in smalls):
        return None
    abin_owner = [e for e in bigs for _ in range(2)] + list(mids)
    bbin_owner = [e for e in smalls for _ in range(2)] + list(mids)
    cbin_owner = bigs + smalls + list(mids)
    expert_bins = {e: [] for e in order}
    for core in range(NCORES):
        expert_bins[abin_owner[core]].append((core, 0, sA))
        expert_bins[bbin_owner[core]].append((core, 1, sB))
        expert_bins[cbin_owner[core]].append((core, 2, sC))
    plan = [[None] * NS for _ in range(NCORES)]
    for e, bins in expert_bins.items():
        lo = 0
        for core, slot, cap in bins:
            hi = min(int(loads[e]), lo + cap)
            plan[core][slot] = (e, lo, hi)
            lo = hi
        if lo != loads[e]:
            return None
    return plan


def _assign(loads):
    """Pack 8 expert loads into 8 cores x 3 segment bins.

    Returns (C, plan) with plan[core][seg] = (expert, tok_lo, tok_hi)
    (token range within that expert's gathered list). Tries the tightest
    capacity first; falls back to classic one-expert-per-core.
    """
    for C in (2112, 2176):
        plan = _try_pack(loads, C)
        if plan is not None:
            return C, plan
    # classic: one expert per core, all 3 segments
    C = max(2176, -(-max(loads) // P) * P)
    plan = []
    for e in range(NCORES):
        lo = 0
        segs = []
        for B in _segs(C):
            hi = min(loads[e], lo + B)
            segs.append((e, lo, hi))
            lo = hi
        plan.append(segs)
    return C, plan


def kernel(x, gate_w, w1, w2, w3, _trace=False, _trace_cores=None, _result_box=None):
    import ml_dtypes

    from concourse.bass_utils import run_bass_kernel_spmd

    bf16 = ml_dtypes.bfloat16
    x = np.ascontiguousarray(np.asarray(x, dtype=np.float32))
    gw = np.ascontiguousarray(np.asarray(gate_w, dtype=np.float32))
    w1 = np.asarray(w1, dtype=np.float32)
    w2 = np.asarray(w2, dtype=np.float32)
    w3 = np.asarray(w3, dtype=np.float32)
    assert x.shape == (T, D) and gw.shape == (D, E), (x.shape, gw.shape)
    assert w1.shape == (E, D, F) and w3.shape == (E, D, F), (w1.shape,)
    assert w2.shape == (E, F, D), (w2.shape,)

    idx = _route(x, gw)
    loads = np.array([len(i) for i in idx])
    C, plan = _assign(loads)

    if C not in _nc_cache:
        _nc_cache[C] = _build(C)
    nc = _nc_cache[C]

    xb = x.astype(bf16)
    rot = np.arange(E)
    # per-expert pre-transposed weights (shared across cores/segments)
    w1T = [
        np.ascontiguousarray(
            w1[e].astype(bf16).reshape(KD, P, KF, P).transpose(2, 1, 0, 3)
        ).reshape(KF, P, KD * P)
        for e in range(E)
    ]
    w3T = [
        np.ascontiguousarray(
            w3[e].astype(bf16).reshape(KD, P, KF, P).transpose(2, 1, 0, 3)
        ).reshape(KF, P, KD * P)
        for e in range(E)
    ]
    w2T = [
        np.ascontiguousarray(
            w2[e].astype(bf16).reshape(2, KH, P, KD, P).transpose(0, 3, 2, 1, 4)
        ).reshape(2, KD, P, KH * P)
        for e in range(E)
    ]
    gwT = [
        np.ascontiguousarray(
            gw[:, (rot + e) % E].astype(bf16).reshape(KD, P, E).transpose(1, 0, 2)
        ).reshape(P, KD * E)
        for e in range(E)
    ]

    selc = np.zeros((4, 32, P), bf16)
    for s in range(4):
        selc[s, s, :] = 1.0

    NB = -(-C // P)
    offs = np.concatenate([[0], np.cumsum(_segs(C))])
    in_maps = []
    for core in range(NCORES):
        xt = np.zeros((P, KD, C), bf16)
        valid_flat = np.zeros(NB * P, np.float32)
        gwt = np.empty((NS, P, KD * E), bf16)
        w1t = np.empty((NS, KF, P, KD * P), bf16)
        w3t = np.empty((NS, KF, P, KD * P), bf16)
        w2t = np.empty((NS, 2, KD, P, KH * P), bf16)
        for s in range(NS):
            e, lo, hi = plan[core][s]
            n = hi - lo
            o = offs[s]
            if n:
                xt[:, :, o : o + n] = (
                    xb[idx[e][lo:hi]].reshape(n, KD, P).transpose(2, 1, 0)
                )
                valid_flat[o : o + n] = 1.0
            gwt[s] = gwT[e]
            w1t[s] = w1T[e]
            w3t[s] = w3T[e]
            w2t[s] = w2T[e]
        valid = np.ascontiguousarray(
            valid_flat.reshape(NB, P).T
        )  # [P, NB], token t=(o*P+p) -> [p, o]
        in_maps.append(
            {
                "xt": xt,
                "gwt": gwt,
                "w1t": w1t,
                "w3t": w3t,
                "w2t": w2t,
                "valid": valid,
                "selc": selc,
            }
        )

    res = run_bass_kernel_spmd(
        nc,
        in_maps,
        core_ids=list(range(NCORES)),
        trace=_trace,
        trace_cores=_trace_cores,
    )
    if _result_box is not None:
        _result_box.append(res)

    out = np.zeros((T, D), np.float32)
    for core in range(NCORES):
        yt = np.asarray(res.results[core]["yt"]).astype(np.float32)
        for s in range(NS):
            e, lo, hi = plan[core][s]
            n = hi - lo
            if n:
                o = offs[s]
                out[idx[e][lo:hi]] += yt[:, o : o + n].T
    return out
